# revision 37
# baseline (speedup 1.0000x reference)
"""Trainium2 Bass kernel for a dense transformer block (pre-LN, MHA + MLP).

Data-parallel over batch: 8 batch elements, one per NeuronCore; weights
replicated, no collectives.

All GEMMs are fp8e4 (e4m3) DoubleRow matmuls (0.5 cycles/row).  Precision
(CPU-validated max scale-rel err ~1.6e-2 vs the 2e-2 gate):
  - attention (QKV proj, scores, P@V, out proj): plain fp8 both operands.
  - MLP1 2-term: (w1h, 32*w1l) x (y2h, y2h/32)  [drops W@y2_lo].
  - MLP2 3-term: (w2h, 32*w2l) x (h8, h8/32) + unscaled hl x w2h.
  - softmax: p = exp(s/8 - log 16) fp8; denominators from an appended
    0.25-scaled ones column in V; reciprocal broadcast via SBUF->SBUF DMA
    and folded into the PSUM->SBUF drain of o^T.
  - LN rstd = exp(-0.5*ln(var+eps)) so softmax-exp, ln and identity share
    one ACT table; gelus batched in a second table epoch (2 loads total).

Schedule: LN1 -> K/Q0/V proj -> [attention q-half 0 | exp epoch] ->
[q-half 1 | exp, with out-proj/LN2/MLP1 of half 0 filling the PE] ->
out-proj/LN2(h1) -> [gelu+MLP2(h0) pipelined per k-block] -> MLP1(h1) ->
[gelu+MLP2(h1)].  MLP2 streams w2 k-major so gelu output feeds matmuls
incrementally; MLP1 stages pre-gelu in bf16 (h_pre) so ACT table epochs
stay contiguous.
"""
import contextlib
import os
import sys

import numpy as np
import ml_dtypes

DEBUG_DUMPS = bool(os.environ.get("BASSDBG"))

sys.path.insert(0, "/opt/trn_rl_repo")

import concourse.bass as bass
import concourse.mybir as mybir
import concourse.tile as tile
from concourse import bacc, bass_utils
from concourse.masks import make_identity

F32 = mybir.dt.float32
F32R = mybir.dt.float32r
BF16 = mybir.dt.bfloat16
FP8 = mybir.dt.float8e4
AF = mybir.ActivationFunctionType
ALU = mybir.AluOpType
DR = mybir.MatmulPerfMode.DoubleRow

P = 128
S = 1024
D = 1024
H = 16
HD = 64
FF = 4096
ST = S // P   # 8
DT = D // P   # 8
FT = FF // P  # 32
NPAIR = H // 2
EPS = 1e-5
NLOG16 = -2.7725887
NO_LO = bool(os.environ.get("NOLO"))


def fap(base, off, dims):
    """AP with base's partition dim, extra element offset, custom free dims."""
    return bass.AP(tensor=base.tensor, offset=base.offset + off,
                   ap=[list(base.ap[0])] + [list(d) for d in dims])


def build_program(zb: bool):
    """zb=True builds the fast path for all-zero biases/unit-free foldings
    (the actual setup_inputs case); zb=False adds the bias instructions."""
    nc = bacc.Bacc("TRN2", target_bir_lowering=False, debug=False)

    x = nc.dram_tensor("x", [S, D], BF16, kind="ExternalInput").ap()
    wqkv8 = nc.dram_tensor("wqkv8", [P, DT, 3 * D], FP8, kind="ExternalInput").ap()
    wop8 = nc.dram_tensor("wop8", [P, NPAIR, D], FP8, kind="ExternalInput").ap()
    wpack1 = nc.dram_tensor("wpack1", [16, P, DT, 2, 256], FP8,
                            kind="ExternalInput").ap()
    w2kpack = nc.dram_tensor("w2kpack", [16, P, 2, 2, D], FP8,
                             kind="ExternalInput").ap()
    qk_bias = nc.dram_tensor("qk_bias", [P, 2, NPAIR], F32, kind="ExternalInput").ap()
    vbias8 = nc.dram_tensor("vbias8", [1, D], FP8, kind="ExternalInput").ap()
    ones_aux8 = nc.dram_tensor("ones_aux8", [1, 2 * P], FP8, kind="ExternalInput").ap()
    bout8 = nc.dram_tensor("bout8", [1, D], FP8, kind="ExternalInput").ap()
    b1_col = nc.dram_tensor("b1_col", [P, FT], F32, kind="ExternalInput").ap()
    b2_8 = nc.dram_tensor("b2_8", [1, D], FP8, kind="ExternalInput").ap()
    out = nc.dram_tensor("out", [S, D], F32, kind="ExternalOutput").ap()
    if DEBUG_DUMPS:
        d_y1 = nc.dram_tensor("d_y1", [P, DT, S], FP8, kind="ExternalOutput").ap()
        d_vext = nc.dram_tensor("d_vext", [P, ST, H, HD + 1], FP8,
                                kind="ExternalOutput").ap()
        d_qT = nc.dram_tensor("d_qT", [P, NPAIR, 2, 512], FP8,
                              kind="ExternalOutput").ap()
        d_kT = nc.dram_tensor("d_kT", [P, NPAIR, S], FP8,
                              kind="ExternalOutput").ap()
        d_oTp = nc.dram_tensor("d_oTp", [P, NPAIR, S], FP8,
                               kind="ExternalOutput").ap()
        d_x2 = nc.dram_tensor("d_x2", [P, ST, D], BF16, kind="ExternalOutput").ap()
        d_y2h = nc.dram_tensor("d_y2h", [P, DT, 2, S], FP8,
                               kind="ExternalOutput").ap()
        d_hpre = nc.dram_tensor("d_hpre", [P, FT, 512], BF16,
                                kind="ExternalOutput").ap()

    with tile.TileContext(nc) as tc, contextlib.ExitStack() as ctx:
        singles = ctx.enter_context(tc.tile_pool(name="singles", bufs=1))
        big = ctx.enter_context(tc.tile_pool(name="big", bufs=1))
        outp = ctx.enter_context(tc.tile_pool(name="outp", bufs=2))
        dram = ctx.enter_context(tc.tile_pool(name="dram", bufs=1, space="DRAM"))

        # ---- constants ----
        ident = singles.tile([P, P], F32)
        make_identity(nc, ident)
        identr = singles.tile([P, P], F32R)
        nc.vector.tensor_copy(out=identr, in_=ident)
        eps_t = singles.tile([P, 1], F32)
        nc.vector.memset(eps_t, EPS)
        nbias_t = singles.tile([P, 1], F32)
        nc.vector.memset(nbias_t, NLOG16)
        c32_t = singles.tile([P, 1], F32)
        nc.vector.memset(c32_t, 1.0 / 32.0)
        cq_t = singles.tile([P, 1], F32)
        nc.vector.memset(cq_t, 0.25)
        cm1_t = singles.tile([P, 1], F32)
        nc.vector.memset(cm1_t, -1.0)
        if not zb:
            onesz = singles.tile([1, 2, P], FP8)
            nc.sync.dma_start(onesz, ones_aux8.rearrange("o (t p) -> o t p", t=2))
            vb8 = singles.tile([1, D], FP8)
            nc.sync.dma_start(vb8, vbias8)
            bo8 = singles.tile([1, D], FP8)
            nc.sync.dma_start(bo8, bout8)
            bb2 = singles.tile([1, D], FP8)
            nc.sync.dma_start(bb2, b2_8)
            qkb = singles.tile([P, 2, NPAIR], F32)
            nc.sync.dma_start(qkb, qk_bias)
        b1c = singles.tile([P, FT], F32)
        nc.sync.dma_start(b1c, b1_col)

        def bias_mm(ps_ap, row8, c0, n, start, stop):
            rhs = fap(row8[0:1], c0, [[0, 2], [1, n]])
            nc.tensor.matmul(ps_ap, lhsT=onesz, rhs=rhs, start=start, stop=stop,
                             perf_mode=DR, skip_group_check=True)

        # ---- persistent tiles (never released) ----
        kT = big.tile([P, NPAIR, S], FP8, tag="kT")          # 8KB
        qT = big.tile([P, NPAIR, 2, 512], FP8, tag="qT")     # 8KB (slot1 zero)
        v_ext = big.tile([P, ST, H, HD + 1], FP8, tag="vx")  # 8.5KB
        oTp = big.tile([P, NPAIR, S], FP8, tag="oTp")        # 8KB
        y2h = big.tile([P, DT, 2, S], FP8, tag="y2h")        # 16KB
        x2 = big.tile([P, ST, D], BF16, tag="x2")            # 16KB
        wo8 = big.tile([P, NPAIR, D], FP8, tag="wo8")        # 8KB
        h_pre = big.tile([P, FT, 512], BF16, tag="hpre")     # 32KB (per half)
        wch = ctx.enter_context(tc.tile_pool(name="wch", bufs=2))

        # mid-lifetime pools: [start, end of D/E half1]
        mid_ctx = contextlib.ExitStack()
        work_ps = mid_ctx.enter_context(
            tc.tile_pool(name="work_ps", bufs=2, space="PSUM"))
        yln = mid_ctx.enter_context(tc.tile_pool(name="yln", bufs=1))
        ln2p = mid_ctx.enter_context(tc.tile_pool(name="ln2p", bufs=4))
        xrp = mid_ctx.enter_context(tc.tile_pool(name="xrp", bufs=2))
        # attention pools
        att_ctx = contextlib.ExitStack()
        ssc_ps = att_ctx.enter_context(
            tc.tile_pool(name="ssc_ps", bufs=2, space="PSUM"))
        ot_ps = att_ctx.enter_context(
            tc.tile_pool(name="ot_ps", bufs=2, space="PSUM"))
        ptp = att_ctx.enter_context(tc.tile_pool(name="ptp", bufs=8))
        rdp = att_ctx.enter_context(tc.tile_pool(name="rdp", bufs=3))
        rbp = att_ctx.enter_context(tc.tile_pool(name="rbp", bufs=2))
        # qkv weights + y1 live only through the projections
        wqy_ctx = contextlib.ExitStack()
        wqy = wqy_ctx.enter_context(tc.tile_pool(name="wqy", bufs=1))
        wq8 = wqy.tile([P, DT, 3 * D], FP8, tag="wq8")       # 24KB
        y1 = wqy.tile([P, DT, S], FP8, tag="y1")             # 8KB
        lnp_ctx = contextlib.ExitStack()
        lnp = lnp_ctx.enter_context(tc.tile_pool(name="lnp", bufs=4))

        nc.sync.dma_start(wq8, wqkv8)
        nc.sync.dma_start(wo8, wop8)
        nc.gpsimd.memset(qT[:, :, 1, :], 0.0)
        nc.gpsimd.memset(v_ext[:, :, :, HD:HD + 1], 0.25)

        # ---------------- LN helpers ----------------
        def ln_rstd(x_row, ln):
            """bn stats + rstd = exp(-0.5 ln(var+eps)); returns (rstd, negms)."""
            stats = ln.tile([P, 2, 6], F32, tag="stats")
            xg = x_row.rearrange("p (n f) -> p n f", f=512)
            for g in range(2):
                nc.vector.bn_stats(out=stats[:, g, :], in_=xg[:, g, :])
            mv = ln.tile([P, 2], F32, tag="mv")
            nc.vector.bn_aggr(out=mv, in_=stats)
            rstd = ln.tile([P, 1], F32, tag="rstd")
            nc.scalar.activation(out=rstd, in_=mv[:, 1:2], func=AF.Ln,
                                 bias=eps_t, scale=1.0)
            nc.scalar.activation(out=rstd, in_=rstd, func=AF.Exp,
                                 bias=0.0, scale=-0.5)
            negms = ln.tile([P, 1], F32, tag="negms")
            nc.vector.tensor_scalar(out=negms, in0=mv[:, 0:1], scalar1=rstd,
                                    scalar2=cm1_t, op0=ALU.mult, op1=ALU.mult)
            return rstd, negms

        def ln_transpose(st, y, yh, slot1):
            """transpose y (f32r row) into feature-major fp8 yh; slot1 -> /32."""
            for dg in range(2):
                ps = work_ps.tile([P, 512], F32, tag="w", name="tp_ps")
                for j in range(4):
                    dt = dg * 4 + j
                    nc.tensor.transpose(
                        ps[:, j * P:(j + 1) * P].bitcast(F32R),
                        y[:, dt * P:(dt + 1) * P], identr)
                if slot1:
                    oap = fap(yh[:, 0, 0, 0:1], (dg * 4) * 2 * S + st * P,
                              [[2 * S, 4], [1, P]])
                    o32 = fap(yh[:, 0, 0, 0:1], (dg * 4) * 2 * S + S + st * P,
                              [[2 * S, 4], [1, P]])
                    nc.vector.tensor_copy(out=oap, in_=ps)
                    nc.vector.tensor_scalar(out=o32, in0=ps, scalar1=c32_t,
                                            scalar2=None, op0=ALU.mult)
                else:
                    oap = fap(yh[:, 0, 0:1], (dg * 4) * S + st * P,
                              [[S, 4], [1, P]])
                    nc.vector.tensor_copy(out=oap, in_=ps)

        # ---------------- Phase A: LN1 ----------------
        for st in range(ST):
            xr = xrp.tile([P, D], BF16, tag="xln", name="xln")
            nc.gpsimd.dma_start(xr, x[st * P:(st + 1) * P, :])
            rstd, negms = ln_rstd(xr, lnp)
            y = yln.tile([P, D], F32R, tag="y", name="yy")
            nc.scalar.activation(out=y, in_=xr, func=AF.Identity,
                                 scale=rstd, bias=negms)
            ln_transpose(st, y, y1, slot1=False)
        lnp_ctx.close()

        # ---------------- projections ----------------
        def proj_feat(col0, dst_part, dst_ap, bias_col):
            ps = work_ps.tile([P, 512], F32, tag="w", name="proj")
            for kp in range(4):
                lhs = fap(wq8[:, 0, 0:1], (2 * kp) * 3 * D + col0,
                          [[3 * D, 2], [1, P]])
                rhs = fap(y1[:, 0, 0:1], (2 * kp) * S + dst_part * 512,
                          [[S, 2], [1, 512]])
                nc.tensor.matmul(ps, lhsT=lhs, rhs=rhs, start=(kp == 0),
                                 stop=(kp == 3), perf_mode=DR,
                                 skip_group_check=True)
            if zb:
                nc.vector.tensor_copy(out=dst_ap, in_=ps)
            else:
                nc.vector.tensor_scalar(out=dst_ap, in0=ps, scalar1=bias_col,
                                        scalar2=None, op0=ALU.add)

        def k_proj(pr):
            for sh in range(2):
                dst = kT[:, pr, sh * 512:(sh + 1) * 512]
                proj_feat(2 * D + pr * P, sh, dst,
                          None if zb else qkb[:, 1, pr:pr + 1])

        def q_proj(pr, qt):
            proj_feat(D + pr * P, qt, qT[:, pr, 0, :],
                      None if zb else qkb[:, 0, pr:pr + 1])

        def v_proj(vc):
            for it in range(ST):
                ps = work_ps.tile([P, 512], F32, tag="w", name="vproj")
                for kp in range(4):
                    lhs = fap(y1[:, 0, 0:1], (2 * kp) * S + it * P,
                              [[S, 2], [1, P]])
                    rhs = fap(wq8[:, 0, 0:1], (2 * kp) * 3 * D + vc * 512,
                              [[3 * D, 2], [1, 512]])
                    nc.tensor.matmul(ps, lhsT=lhs, rhs=rhs, start=(kp == 0),
                                     stop=zb and (kp == 3), perf_mode=DR,
                                     skip_group_check=True)
                if not zb:
                    bias_mm(ps[:, 0:256], vb8, vc * 512, 256, False, False)
                    bias_mm(ps[:, 256:512], vb8, vc * 512 + 256, 256, False, True)
                oap = fap(v_ext[:, 0, 0, 0:1], it * H * (HD + 1) + vc * 8 * (HD + 1),
                          [[HD + 1, 8], [1, HD]])
                nc.vector.tensor_scalar(out=oap,
                                        in0=ps.rearrange("p (h c) -> p h c", c=HD),
                                        scalar1=cq_t, scalar2=None, op0=ALU.mult)

        for pr in range(NPAIR):
            k_proj(pr)
        for pr in range(NPAIR):
            q_proj(pr, 0)

        # ---------------- attention ----------------
        recip_dram = dram.tile([H, 2, 512], F32)

        def attn_pair(p, qt):
            """scores/exp/PV/recip/normalized-drain for head pair p, half qt."""
            pts = []
            for e in range(2):
                for jh in range(4):      # key-tile groups of 2
                    ssc = ssc_ps.tile([P, 2, 512], F32, tag="ssc")
                    for jj in range(2):
                        jt = jh * 2 + jj
                        lhs = fap(kT[e * HD:(e + 1) * HD, 0, 0:1],
                                  p * S + jt * P, [[0, 2], [1, P]])
                        rhs = fap(qT[e * HD:(e + 1) * HD, 0, 0, 0:1], p * 1024,
                                  [[512, 2], [1, 512]])
                        nc.tensor.matmul(ssc[:, jj, :], lhsT=lhs, rhs=rhs,
                                         start=True, stop=True,
                                         perf_mode=DR, skip_group_check=True)
                    pt = ptp.tile([P, 2, 512], FP8, tag="pT")
                    nc.scalar.activation(out=pt, in_=ssc, func=AF.Exp,
                                         scale=0.125, bias=nbias_t)
                    pts.append(pt)
            for e in range(2):
                ot = ot_ps.tile([HD + 1, 512], F32, tag="ot")
                h = 2 * p + e
                for jc in range(4):
                    lhs = fap(v_ext[:, 0, 0, 0:1],
                              (jc * 2) * H * (HD + 1) + h * (HD + 1),
                              [[H * (HD + 1), 2], [1, HD + 1]])
                    rhs = fap(pts[4 * e + jc][:, 0, 0:1], 0,
                              [[512, 2], [1, 512]])
                    nc.tensor.matmul(ot, lhsT=lhs, rhs=rhs, start=(jc == 0),
                                     stop=(jc == 3), perf_mode=DR,
                                     skip_group_check=True)
                rd = rdp.tile([1, 512], F32, tag="rd")
                nc.vector.reciprocal(out=rd, in_=ot[HD:HD + 1, :])
                nc.sync.dma_start(
                    out=recip_dram.bitcast(F32)[h:h + 1, qt, :], in_=rd)
                rh, hsl = h // 8, h % 8
                dst = oTp[rh * HD:(rh + 1) * HD, hsl, qt * 512:(qt + 1) * 512]
                nc.vector.tensor_copy(out=dst, in_=ot[0:HD, :])

        def normalize_group(qt, rh):
            """oTp[rh half, :, qt half] *= recip broadcast (8 heads batched)."""
            rbc = rbp.tile([HD, NPAIR, 512], F32, tag="rbc", name="rbc")
            nc.sync.dma_start(out=rbc, in_=bass.AP(
                tensor=recip_dram.tensor,
                offset=recip_dram.offset + (rh * 8 * 2 + qt) * 512,
                ap=[[0, HD], [2 * 512, NPAIR], [1, 512]]))
            sl = oTp[rh * HD:(rh + 1) * HD, :, qt * 512:(qt + 1) * 512]
            nc.vector.tensor_tensor(out=sl, in0=sl, in1=rbc, op=ALU.mult)

        # ---- D/E helpers ----
        def outproj_rows(it):
            """x2 row it = x + oTp @ wo."""
            for ct in range(2):
                ps = work_ps.tile([P, 512], F32, tag="w", name="opj")
                for hp in range(4):
                    lhs = fap(oTp[:, 0, 0:1], (2 * hp) * S + it * P,
                              [[S, 2], [1, P]])
                    rhs = fap(wo8[:, 0, 0:1], (2 * hp) * D + ct * 512,
                              [[D, 2], [1, 512]])
                    nc.tensor.matmul(ps, lhsT=lhs, rhs=rhs, start=(hp == 0),
                                     stop=zb and (hp == 3), perf_mode=DR,
                                     skip_group_check=True)
                if not zb:
                    bias_mm(ps[:, 0:256], bo8, ct * 512, 256, False, False)
                    bias_mm(ps[:, 256:512], bo8, ct * 512 + 256, 256, False, True)
                xr = xrp.tile([P, 512], BF16, tag="xr2", name="xr2")
                nc.gpsimd.dma_start(
                    xr, x[it * P:(it + 1) * P, ct * 512:(ct + 1) * 512])
                nc.vector.tensor_add(out=x2[:, it, ct * 512:(ct + 1) * 512],
                                     in0=ps, in1=xr)

        def ln2_row(it):
            rstd, negms = ln_rstd(x2[:, it, :], ln2p)
            y = yln.tile([P, D], F32R, tag="y", name="yy")
            nc.vector.tensor_scalar(out=y, in0=x2[:, it, :], scalar1=rstd,
                                    scalar2=negms, op0=ALU.mult, op1=ALU.add)
            ln_transpose(it, y, y2h, slot1=True)

        def mlp1_half_iter(sh, ps_pool):
            """MLP1 for all 32 ff blocks of half sh (streams w1); yields
            after each block so callers can interleave emission."""
            w1c = None
            for ft in range(FT):
                if ft % 2 == 0:
                    w1c = wch.tile([P, DT, 2, 256], FP8, tag="w1c", name="w1c")
                    nc.sync.dma_start(w1c, wpack1[ft // 2])
                ps = ps_pool.tile([P, 512], F32, tag="w", name="m1")
                for kt in range(DT):
                    lhsA = fap(w1c[:, 0, 0, 0:1], kt * 512 + (ft % 2) * P,
                               [[256, 2], [1, P]])
                    rhsA = fap(y2h[:, 0, 0, 0:1], kt * 2 * S + sh * 512,
                               [[S, 2], [1, 512]])
                    nc.tensor.matmul(ps, lhsT=lhsA, rhs=rhsA, start=(kt == 0),
                                     stop=(kt == DT - 1), perf_mode=DR,
                                     skip_group_check=True)
                nc.gpsimd.tensor_copy(out=h_pre[:, ft, :], in_=ps)
                yield ft

        def mlp1_half(sh, ps_pool):
            for _ in mlp1_half_iter(sh, ps_pool):
                pass

        # ======== q-half 0 epoch ========
        v_proj(0)           # heads 0-7: covers PV of pairs 0-3
        attn_pair(0, 0)
        v_proj(1)           # heads 8-15
        attn_pair(1, 0)
        for p in range(2, NPAIR):
            attn_pair(p, 0)
            if p == 3:
                normalize_group(0, 0)
            q_proj(p - 2, 1)
        normalize_group(0, 1)
        if DEBUG_DUMPS:
            nc.sync.dma_start(d_y1, y1)
            nc.sync.dma_start(d_vext, v_ext)
            nc.sync.dma_start(d_kT, kT)
            nc.sync.dma_start(d_qT, qT)
        q_proj(6, 1)
        q_proj(7, 1)
        wqy_ctx.close()

        # ======== q-half 1 epoch: D/E(h0) first, MLP1(h0) interleaved ====
        mlp1_iter = mlp1_half_iter(0, work_ps)
        for p in range(NPAIR):
            attn_pair(p, 1)
            if p == 0:
                for it in range(4):
                    outproj_rows(it)
                    ln2_row(it)
            else:
                for _ in range(5):
                    next(mlp1_iter, None)
            if p == 3:
                normalize_group(1, 0)
        normalize_group(1, 1)
        att_ctx.close()

        if DEBUG_DUMPS:
            nc.sync.dma_start(d_oTp, oTp)
        # D/E half 1 rows; finish any remaining MLP1(half0) blocks
        for it in range(4, ST):
            outproj_rows(it)
            ln2_row(it)
        for _ in mlp1_iter:
            pass
        if DEBUG_DUMPS:
            nc.sync.dma_start(d_x2, x2)
            nc.sync.dma_start(d_y2h, y2h)
            nc.sync.dma_start(d_hpre, h_pre)
        mid_ctx.close()

        # ======== gelu + MLP2 epochs ========
        tail_ctx = contextlib.ExitStack()
        h1p = tail_ctx.enter_context(tc.tile_pool(name="h1p", bufs=3))
        w2p = tail_ctx.enter_context(tc.tile_pool(name="w2p", bufs=3))
        hgp = tail_ctx.enter_context(tc.tile_pool(name="hgp", bufs=3))

        def mlp2_half(sh):
            with tc.tile_pool(name="m2_ps", bufs=1, space="PSUM") as m2_ps:
                tiles = [m2_ps.tile([P, 512], F32, tag=f"m2_{i}", name=f"m2_{i}")
                         for i in range(8)]
                if not zb:
                    for i, tl in enumerate(tiles):
                        il, cth = i // 2, i % 2
                        bias_mm(tl[:, 0:256], bb2, cth * 512, 256, True, False)
                        bias_mm(tl[:, 256:512], bb2, cth * 512 + 256, 256,
                                False, False)
                for bk in range(16):
                    w2k = w2p.tile([P, 2, 2, D], FP8, tag="w2k", name="w2k")
                    nc.gpsimd.dma_start(w2k, w2kpack[bk])
                    h1t = h1p.tile([P, 2, 3, 512], FP8, tag="h1t", name="h1t")
                    hg = hgp.tile([P, 2, 512], BF16, tag="hg", name="hg")
                    # gelu (bf16): 2 ff-blocks from h_pre
                    if zb:
                        gin = fap(h_pre[:, 0, 0:1], (2 * bk) * 512, [[1, 1024]])
                        nc.scalar.activation(out=hg, in_=gin, func=AF.Gelu,
                                             bias=0.0, scale=1.0)
                    else:
                        for k2 in range(2):
                            nc.scalar.activation(
                                out=hg[:, k2, :],
                                in_=h_pre[:, 2 * bk + k2, :], func=AF.Gelu,
                                bias=b1c[:, 2 * bk + k2:2 * bk + k2 + 1],
                                scale=1.0)
                    for k2 in range(2):
                        nc.vector.tensor_copy(out=h1t[:, k2, 0, :],
                                              in_=hg[:, k2, :])
                        nc.vector.tensor_scalar(out=h1t[:, k2, 1, :],
                                                in0=h1t[:, k2, 0, :],
                                                scalar1=c32_t, scalar2=None,
                                                op0=ALU.mult)
                        nc.vector.scalar_tensor_tensor(
                            out=h1t[:, k2, 2, :], in0=h1t[:, k2, 0, :],
                            scalar=-1.0, in1=hg[:, k2, :],
                            op0=ALU.mult, op1=ALU.add)
                    for i, tl in enumerate(tiles):
                        il, cth = i // 2, i % 2
                        for k2 in range(2):
                            lhsA = fap(h1t[:, 0, 0, 0:1], k2 * 1536 + il * P,
                                       [[512, 2], [1, P]])
                            rhsA = fap(w2k[:, 0, 0, 0:1], k2 * 2 * D + cth * 512,
                                       [[D, 2], [1, 512]])
                            nc.tensor.matmul(
                                tl, lhsT=lhsA, rhs=rhsA,
                                start=(zb and bk == 0 and k2 == 0),
                                stop=(bk == 15 and k2 == 1 and NO_LO),
                                perf_mode=DR, skip_group_check=True)
                        if not NO_LO:
                            lhsB = fap(h1t[:, 0, 0, 0:1], 2 * 512 + il * P,
                                       [[1536, 2], [1, P]])
                            rhsB = fap(w2k[:, 0, 0, 0:1], cth * 512,
                                       [[2 * D, 2], [1, 512]])
                            nc.tensor.matmul(tl, lhsT=lhsB, rhs=rhsB, start=False,
                                             stop=(bk == 15), perf_mode=DR,
                                             skip_group_check=True)
                for i, tl in enumerate(tiles):
                    il, cth = i // 2, i % 2
                    it = sh * 4 + il
                    ot = outp.tile([P, 512], F32, tag="fin")
                    nc.vector.tensor_add(
                        out=ot, in0=tl,
                        in1=x2[:, it, cth * 512:(cth + 1) * 512])
                    nc.sync.dma_start(
                        out=out[it * P:(it + 1) * P, cth * 512:(cth + 1) * 512],
                        in_=ot)

        mlp2_half(0)
        with tc.tile_pool(name="m1b_ps", bufs=2, space="PSUM") as m1b_ps:
            mlp1_half(1, m1b_ps)
        mlp2_half(1)
        tail_ctx.close()

    nc.compile()
    return nc


_NC_CACHE = {}


def _get_nc(zb=True):
    if zb not in _NC_CACHE:
        _NC_CACHE[zb] = build_program(zb)
    return _NC_CACHE[zb]


def _q8(a):
    return a.astype(ml_dtypes.float8_e4m3)


def _prep_weights(inputs):
    f32 = lambda k: np.asarray(inputs[k], np.float32)
    ln1_g, ln1_b = f32("ln1_g"), f32("ln1_b")
    ln2_g, ln2_b = f32("ln2_g"), f32("ln2_b")
    w_qkv, w_out, b_out = f32("w_qkv"), f32("w_out"), f32("b_out")
    w1, b1, w2, b2 = f32("w1"), f32("b1"), f32("w2"), f32("b2")

    wqkv_g = w_qkv * ln1_g[:, None]
    wqkv8 = np.ascontiguousarray(
        _q8(wqkv_g).reshape(DT, P, 3 * D).transpose(1, 0, 2))
    bias_qkv = ln1_b @ w_qkv
    qk_bias = np.empty((P, 2, NPAIR), np.float32)
    for pp in range(NPAIR):
        qk_bias[:, 0, pp] = bias_qkv[D + pp * P:D + (pp + 1) * P]
        qk_bias[:, 1, pp] = bias_qkv[2 * D + pp * P:2 * D + (pp + 1) * P]
    vbias8 = _q8(bias_qkv[:D]).reshape(1, D)
    ones_aux = np.zeros((1, 2 * P), np.float32)
    ones_aux[0, :P] = 1.0

    # out-proj, head-pair stacked: wop8[p, hp, :] = w_out[row], where
    # partition p<64 -> head hp feat p ; p>=64 -> head hp+8 feat p-64.
    wr = _q8(w_out).reshape(H, HD, D)
    wop8 = np.ascontiguousarray(np.concatenate(
        [wr[0:8].transpose(1, 0, 2), wr[8:16].transpose(1, 0, 2)],
        axis=0))  # [128, 8, D]

    w1_g = w1 * ln2_g[:, None]
    w1h = _q8(w1_g)
    w1l = _q8(32.0 * (w1_g - w1h.astype(np.float32)))
    # [fc 16, P part, DT kt, 2 (hi, 32*lo), 256] chunk-contiguous
    wpack1 = np.ascontiguousarray(np.stack(
        [w1h.reshape(DT, P, 16, 256).transpose(2, 1, 0, 3),
         w1l.reshape(DT, P, 16, 256).transpose(2, 1, 0, 3)], axis=3))
    bias1 = ln2_b @ w1 + b1
    b1_col = np.ascontiguousarray(bias1.reshape(FT, P).T)

    w2h = _q8(w2).reshape(FT, P, D)
    w2l = _q8(32.0 * (w2 - _q8(w2).astype(np.float32))).reshape(FT, P, D)
    # [bk 16, P, kt2 2, slot 2, D] k-pair-chunk contiguous
    w2kpack = np.ascontiguousarray(
        np.stack([w2h, w2l], axis=1).reshape(16, 2, 2, P, D)
        .transpose(0, 3, 1, 2, 4))

    return {
        "wqkv8": wqkv8, "qk_bias": qk_bias, "vbias8": vbias8,
        "ones_aux8": _q8(ones_aux), "wop8": wop8,
        "bout8": _q8(b_out).reshape(1, D), "b1_col": b1_col,
        "wpack1": wpack1, "w2kpack": w2kpack,
        "b2_8": _q8(b2).reshape(1, D),
    }, (not np.any(bias_qkv) and not np.any(b_out) and not np.any(bias1)
        and not np.any(b2))


WEIGHT_NAMES = ["wqkv8", "qk_bias", "vbias8", "ones_aux8", "wop8", "bout8",
                "b1_col", "wpack1", "w2kpack", "b2_8"]


def kernel(**inputs) -> np.ndarray:
    x = np.asarray(inputs["x"], dtype=np.float32).astype(ml_dtypes.bfloat16)
    B = x.shape[0]
    weights, zb = _prep_weights(inputs)
    nc = _get_nc(zb)
    in_maps = [{"x": np.ascontiguousarray(x[b]), **weights} for b in range(B)]
    res = bass_utils.run_bass_kernel_spmd(nc, in_maps, core_ids=list(range(B)))
    return np.stack([res.results[b]["out"] for b in range(B)], axis=0)


# revision 39
# speedup vs baseline: 1.0007x; 1.0007x over previous
"""Trainium2 Bass kernel for a dense transformer block (pre-LN, MHA + MLP).

Data-parallel over batch: 8 batch elements, one per NeuronCore; weights
replicated, no collectives.

All GEMMs are fp8e4 (e4m3) DoubleRow matmuls (0.5 cycles/row).  Precision
(CPU-validated max scale-rel err ~1.6e-2 vs the 2e-2 gate):
  - attention (QKV proj, scores, P@V, out proj): plain fp8 both operands.
  - MLP1 2-term: (w1h, 32*w1l) x (y2h, y2h/32)  [drops W@y2_lo].
  - MLP2 3-term: (w2h, 32*w2l) x (h8, h8/32) + unscaled hl x w2h.
  - softmax: p = exp(s/8 - log 16) fp8; denominators from an appended
    0.25-scaled ones column in V; reciprocal broadcast via SBUF->SBUF DMA
    and folded into the PSUM->SBUF drain of o^T.
  - LN rstd = exp(-0.5*ln(var+eps)) so softmax-exp, ln and identity share
    one ACT table; gelus batched in a second table epoch (2 loads total).

Schedule: LN1 -> K/Q0/V proj -> [attention q-half 0 | exp epoch] ->
[q-half 1 | exp, with out-proj/LN2/MLP1 of half 0 filling the PE] ->
out-proj/LN2(h1) -> [gelu+MLP2(h0) pipelined per k-block] -> MLP1(h1) ->
[gelu+MLP2(h1)].  MLP2 streams w2 k-major so gelu output feeds matmuls
incrementally; MLP1 stages pre-gelu in bf16 (h_pre) so ACT table epochs
stay contiguous.
"""
import contextlib
import os
import sys

import numpy as np
import ml_dtypes

DEBUG_DUMPS = bool(os.environ.get("BASSDBG"))

sys.path.insert(0, "/opt/trn_rl_repo")

import concourse.bass as bass
import concourse.mybir as mybir
import concourse.tile as tile
from concourse import bacc, bass_utils
from concourse.masks import make_identity

F32 = mybir.dt.float32
F32R = mybir.dt.float32r
BF16 = mybir.dt.bfloat16
FP8 = mybir.dt.float8e4
AF = mybir.ActivationFunctionType
ALU = mybir.AluOpType
DR = mybir.MatmulPerfMode.DoubleRow

P = 128
S = 1024
D = 1024
H = 16
HD = 64
FF = 4096
ST = S // P   # 8
DT = D // P   # 8
FT = FF // P  # 32
NPAIR = H // 2
EPS = 1e-5
NLOG16 = -2.7725887
NO_LO = bool(os.environ.get("NOLO"))


def fap(base, off, dims):
    """AP with base's partition dim, extra element offset, custom free dims."""
    return bass.AP(tensor=base.tensor, offset=base.offset + off,
                   ap=[list(base.ap[0])] + [list(d) for d in dims])


def build_program(zb: bool):
    """zb=True builds the fast path for all-zero biases/unit-free foldings
    (the actual setup_inputs case); zb=False adds the bias instructions."""
    nc = bacc.Bacc("TRN2", target_bir_lowering=False, debug=False)

    x = nc.dram_tensor("x", [S, D], BF16, kind="ExternalInput").ap()
    wqkv8 = nc.dram_tensor("wqkv8", [P, DT, 3 * D], FP8, kind="ExternalInput").ap()
    wop8 = nc.dram_tensor("wop8", [P, NPAIR, D], FP8, kind="ExternalInput").ap()
    wpack1 = nc.dram_tensor("wpack1", [16, P, DT, 2, 256], FP8,
                            kind="ExternalInput").ap()
    w2kpack = nc.dram_tensor("w2kpack", [16, P, 2, 2, D], FP8,
                             kind="ExternalInput").ap()
    qk_bias = nc.dram_tensor("qk_bias", [P, 2, NPAIR], F32, kind="ExternalInput").ap()
    vbias8 = nc.dram_tensor("vbias8", [1, D], FP8, kind="ExternalInput").ap()
    ones_aux8 = nc.dram_tensor("ones_aux8", [1, 2 * P], FP8, kind="ExternalInput").ap()
    bout8 = nc.dram_tensor("bout8", [1, D], FP8, kind="ExternalInput").ap()
    b1_col = nc.dram_tensor("b1_col", [P, FT], F32, kind="ExternalInput").ap()
    b2_8 = nc.dram_tensor("b2_8", [1, D], FP8, kind="ExternalInput").ap()
    out = nc.dram_tensor("out", [S, D], F32, kind="ExternalOutput").ap()
    if DEBUG_DUMPS:
        d_y1 = nc.dram_tensor("d_y1", [P, DT, S], FP8, kind="ExternalOutput").ap()
        d_vext = nc.dram_tensor("d_vext", [P, ST, H, HD + 1], FP8,
                                kind="ExternalOutput").ap()
        d_qT = nc.dram_tensor("d_qT", [P, NPAIR, 2, 512], FP8,
                              kind="ExternalOutput").ap()
        d_kT = nc.dram_tensor("d_kT", [P, NPAIR, S], FP8,
                              kind="ExternalOutput").ap()
        d_oTp = nc.dram_tensor("d_oTp", [P, NPAIR, S], FP8,
                               kind="ExternalOutput").ap()
        d_x2 = nc.dram_tensor("d_x2", [P, ST, D], BF16, kind="ExternalOutput").ap()
        d_y2h = nc.dram_tensor("d_y2h", [P, DT, 2, S], FP8,
                               kind="ExternalOutput").ap()
        d_hpre = nc.dram_tensor("d_hpre", [P, FT, 512], BF16,
                                kind="ExternalOutput").ap()

    with tile.TileContext(nc) as tc, contextlib.ExitStack() as ctx:
        singles = ctx.enter_context(tc.tile_pool(name="singles", bufs=1))
        big = ctx.enter_context(tc.tile_pool(name="big", bufs=1))
        outp = ctx.enter_context(tc.tile_pool(name="outp", bufs=2))
        dram = ctx.enter_context(tc.tile_pool(name="dram", bufs=1, space="DRAM"))

        # ---- constants ----
        ident = singles.tile([P, P], F32)
        make_identity(nc, ident)
        identr = singles.tile([P, P], F32R)
        nc.vector.tensor_copy(out=identr, in_=ident)
        eps_t = singles.tile([P, 1], F32)
        nc.vector.memset(eps_t, EPS)
        nbias_t = singles.tile([P, 1], F32)
        nc.vector.memset(nbias_t, NLOG16)
        c32_t = singles.tile([P, 1], F32)
        nc.vector.memset(c32_t, 1.0 / 32.0)
        cq_t = singles.tile([P, 1], F32)
        nc.vector.memset(cq_t, 0.25)
        cm1_t = singles.tile([P, 1], F32)
        nc.vector.memset(cm1_t, -1.0)
        if not zb:
            onesz = singles.tile([1, 2, P], FP8)
            nc.sync.dma_start(onesz, ones_aux8.rearrange("o (t p) -> o t p", t=2))
            vb8 = singles.tile([1, D], FP8)
            nc.sync.dma_start(vb8, vbias8)
            bo8 = singles.tile([1, D], FP8)
            nc.sync.dma_start(bo8, bout8)
            bb2 = singles.tile([1, D], FP8)
            nc.sync.dma_start(bb2, b2_8)
            qkb = singles.tile([P, 2, NPAIR], F32)
            nc.sync.dma_start(qkb, qk_bias)
        b1c = singles.tile([P, FT], F32)
        nc.sync.dma_start(b1c, b1_col)

        def bias_mm(ps_ap, row8, c0, n, start, stop):
            rhs = fap(row8[0:1], c0, [[0, 2], [1, n]])
            nc.tensor.matmul(ps_ap, lhsT=onesz, rhs=rhs, start=start, stop=stop,
                             perf_mode=DR, skip_group_check=True)

        # ---- persistent tiles (never released) ----
        kT = big.tile([P, NPAIR, S], FP8, tag="kT")          # 8KB
        qT = big.tile([P, NPAIR, 2, 512], FP8, tag="qT")     # 8KB (slot1 zero)
        v_ext = big.tile([P, ST, H, HD + 1], FP8, tag="vx")  # 8.5KB
        oTp = big.tile([P, NPAIR, S], FP8, tag="oTp")        # 8KB
        y2h = big.tile([P, DT, 2, S], FP8, tag="y2h")        # 16KB
        x2 = big.tile([P, ST, D], BF16, tag="x2")            # 16KB
        wo8 = big.tile([P, NPAIR, D], FP8, tag="wo8")        # 8KB
        h_pre = big.tile([P, FT, 512], BF16, tag="hpre")     # 32KB (per half)
        wch = ctx.enter_context(tc.tile_pool(name="wch", bufs=2))

        # mid-lifetime pools: [start, end of D/E half1]
        mid_ctx = contextlib.ExitStack()
        work_ps = mid_ctx.enter_context(
            tc.tile_pool(name="work_ps", bufs=2, space="PSUM"))
        yln = mid_ctx.enter_context(tc.tile_pool(name="yln", bufs=2))
        ln2p = mid_ctx.enter_context(tc.tile_pool(name="ln2p", bufs=4))
        xrp = mid_ctx.enter_context(tc.tile_pool(name="xrp", bufs=2))
        # attention pools
        att_ctx = contextlib.ExitStack()
        ssc_ps = att_ctx.enter_context(
            tc.tile_pool(name="ssc_ps", bufs=2, space="PSUM"))
        ot_ps = att_ctx.enter_context(
            tc.tile_pool(name="ot_ps", bufs=2, space="PSUM"))
        ptp = att_ctx.enter_context(tc.tile_pool(name="ptp", bufs=8))
        rdp = att_ctx.enter_context(tc.tile_pool(name="rdp", bufs=3))
        rbp = att_ctx.enter_context(tc.tile_pool(name="rbp", bufs=1))
        # qkv weights + y1 live only through the projections
        wqy_ctx = contextlib.ExitStack()
        wqy = wqy_ctx.enter_context(tc.tile_pool(name="wqy", bufs=1))
        wq8 = wqy.tile([P, DT, 3 * D], FP8, tag="wq8")       # 24KB
        y1 = wqy.tile([P, DT, S], FP8, tag="y1")             # 8KB
        lnp_ctx = contextlib.ExitStack()
        lnp = lnp_ctx.enter_context(tc.tile_pool(name="lnp", bufs=4))

        nc.sync.dma_start(wq8, wqkv8)
        nc.sync.dma_start(wo8, wop8)
        nc.gpsimd.memset(qT[:, :, 1, :], 0.0)
        nc.gpsimd.memset(v_ext[:, :, :, HD:HD + 1], 0.25)

        # ---------------- LN helpers ----------------
        def ln_rstd(x_row, ln):
            """bn stats + rstd = exp(-0.5 ln(var+eps)); returns (rstd, negms)."""
            stats = ln.tile([P, 2, 6], F32, tag="stats")
            xg = x_row.rearrange("p (n f) -> p n f", f=512)
            for g in range(2):
                nc.vector.bn_stats(out=stats[:, g, :], in_=xg[:, g, :])
            mv = ln.tile([P, 2], F32, tag="mv")
            nc.vector.bn_aggr(out=mv, in_=stats)
            rstd = ln.tile([P, 1], F32, tag="rstd")
            nc.scalar.activation(out=rstd, in_=mv[:, 1:2], func=AF.Ln,
                                 bias=eps_t, scale=1.0)
            nc.scalar.activation(out=rstd, in_=rstd, func=AF.Exp,
                                 bias=0.0, scale=-0.5)
            negms = ln.tile([P, 1], F32, tag="negms")
            nc.vector.tensor_scalar(out=negms, in0=mv[:, 0:1], scalar1=rstd,
                                    scalar2=cm1_t, op0=ALU.mult, op1=ALU.mult)
            return rstd, negms

        def ln_transpose(st, y, yh, slot1):
            """transpose y (f32r row) into feature-major fp8 yh; slot1 -> /32."""
            for dg in range(2):
                ps = work_ps.tile([P, 512], F32, tag="w", name="tp_ps")
                for j in range(4):
                    dt = dg * 4 + j
                    nc.tensor.transpose(
                        ps[:, j * P:(j + 1) * P].bitcast(F32R),
                        y[:, dt * P:(dt + 1) * P], identr)
                if slot1:
                    oap = fap(yh[:, 0, 0, 0:1], (dg * 4) * 2 * S + st * P,
                              [[2 * S, 4], [1, P]])
                    o32 = fap(yh[:, 0, 0, 0:1], (dg * 4) * 2 * S + S + st * P,
                              [[2 * S, 4], [1, P]])
                    nc.vector.tensor_copy(out=oap, in_=ps)
                    nc.vector.tensor_scalar(out=o32, in0=ps, scalar1=c32_t,
                                            scalar2=None, op0=ALU.mult)
                else:
                    oap = fap(yh[:, 0, 0:1], (dg * 4) * S + st * P,
                              [[S, 4], [1, P]])
                    nc.vector.tensor_copy(out=oap, in_=ps)

        # ---------------- Phase A: LN1 ----------------
        for st in range(ST):
            xr = xrp.tile([P, D], BF16, tag="xln", name="xln")
            nc.gpsimd.dma_start(xr, x[st * P:(st + 1) * P, :])
            rstd, negms = ln_rstd(xr, lnp)
            y = yln.tile([P, D], F32R, tag="y", name="yy")
            nc.scalar.activation(out=y, in_=xr, func=AF.Identity,
                                 scale=rstd, bias=negms)
            ln_transpose(st, y, y1, slot1=False)
        lnp_ctx.close()

        # ---------------- projections ----------------
        def proj_feat(col0, dst_part, dst_ap, bias_col):
            ps = work_ps.tile([P, 512], F32, tag="w", name="proj")
            for kp in range(4):
                lhs = fap(wq8[:, 0, 0:1], (2 * kp) * 3 * D + col0,
                          [[3 * D, 2], [1, P]])
                rhs = fap(y1[:, 0, 0:1], (2 * kp) * S + dst_part * 512,
                          [[S, 2], [1, 512]])
                nc.tensor.matmul(ps, lhsT=lhs, rhs=rhs, start=(kp == 0),
                                 stop=(kp == 3), perf_mode=DR,
                                 skip_group_check=True)
            if zb:
                nc.vector.tensor_copy(out=dst_ap, in_=ps)
            else:
                nc.vector.tensor_scalar(out=dst_ap, in0=ps, scalar1=bias_col,
                                        scalar2=None, op0=ALU.add)

        def k_proj(pr):
            for sh in range(2):
                dst = kT[:, pr, sh * 512:(sh + 1) * 512]
                proj_feat(2 * D + pr * P, sh, dst,
                          None if zb else qkb[:, 1, pr:pr + 1])

        def q_proj(pr, qt):
            proj_feat(D + pr * P, qt, qT[:, pr, 0, :],
                      None if zb else qkb[:, 0, pr:pr + 1])

        def v_proj(vc):
            for it in range(ST):
                ps = work_ps.tile([P, 512], F32, tag="w", name="vproj")
                for kp in range(4):
                    lhs = fap(y1[:, 0, 0:1], (2 * kp) * S + it * P,
                              [[S, 2], [1, P]])
                    rhs = fap(wq8[:, 0, 0:1], (2 * kp) * 3 * D + vc * 512,
                              [[3 * D, 2], [1, 512]])
                    nc.tensor.matmul(ps, lhsT=lhs, rhs=rhs, start=(kp == 0),
                                     stop=zb and (kp == 3), perf_mode=DR,
                                     skip_group_check=True)
                if not zb:
                    bias_mm(ps[:, 0:256], vb8, vc * 512, 256, False, False)
                    bias_mm(ps[:, 256:512], vb8, vc * 512 + 256, 256, False, True)
                oap = fap(v_ext[:, 0, 0, 0:1], it * H * (HD + 1) + vc * 8 * (HD + 1),
                          [[HD + 1, 8], [1, HD]])
                nc.vector.tensor_scalar(out=oap,
                                        in0=ps.rearrange("p (h c) -> p h c", c=HD),
                                        scalar1=cq_t, scalar2=None, op0=ALU.mult)

        for pr in range(NPAIR):
            k_proj(pr)
        for pr in range(NPAIR):
            q_proj(pr, 0)

        # ---------------- attention ----------------
        recip_dram = dram.tile([H, 2, 512], F32)

        def attn_pair(p, qt):
            """scores/exp/PV/recip/normalized-drain for head pair p, half qt."""
            pts = []
            for e in range(2):
                for jh in range(4):      # key-tile groups of 2
                    ssc = ssc_ps.tile([P, 2, 512], F32, tag="ssc")
                    for jj in range(2):
                        jt = jh * 2 + jj
                        lhs = fap(kT[e * HD:(e + 1) * HD, 0, 0:1],
                                  p * S + jt * P, [[0, 2], [1, P]])
                        rhs = fap(qT[e * HD:(e + 1) * HD, 0, 0, 0:1], p * 1024,
                                  [[512, 2], [1, 512]])
                        nc.tensor.matmul(ssc[:, jj, :], lhsT=lhs, rhs=rhs,
                                         start=True, stop=True,
                                         perf_mode=DR, skip_group_check=True)
                    pt = ptp.tile([P, 2, 512], FP8, tag="pT")
                    nc.scalar.activation(out=pt, in_=ssc, func=AF.Exp,
                                         scale=0.125, bias=nbias_t)
                    pts.append(pt)
            for e in range(2):
                ot = ot_ps.tile([HD + 1, 512], F32, tag="ot")
                h = 2 * p + e
                for jc in range(4):
                    lhs = fap(v_ext[:, 0, 0, 0:1],
                              (jc * 2) * H * (HD + 1) + h * (HD + 1),
                              [[H * (HD + 1), 2], [1, HD + 1]])
                    rhs = fap(pts[4 * e + jc][:, 0, 0:1], 0,
                              [[512, 2], [1, 512]])
                    nc.tensor.matmul(ot, lhsT=lhs, rhs=rhs, start=(jc == 0),
                                     stop=(jc == 3), perf_mode=DR,
                                     skip_group_check=True)
                rd = rdp.tile([1, 512], F32, tag="rd")
                nc.vector.reciprocal(out=rd, in_=ot[HD:HD + 1, :])
                nc.sync.dma_start(
                    out=recip_dram.bitcast(F32)[h:h + 1, qt, :], in_=rd)
                rh, hsl = h // 8, h % 8
                dst = oTp[rh * HD:(rh + 1) * HD, hsl, qt * 512:(qt + 1) * 512]
                nc.vector.tensor_copy(out=dst, in_=ot[0:HD, :])

        def normalize_group(qt, rh):
            """oTp[rh half, :, qt half] *= recip broadcast (8 heads batched)."""
            rbc = rbp.tile([HD, NPAIR, 512], F32, tag="rbc", name="rbc")
            nc.sync.dma_start(out=rbc, in_=bass.AP(
                tensor=recip_dram.tensor,
                offset=recip_dram.offset + (rh * 8 * 2 + qt) * 512,
                ap=[[0, HD], [2 * 512, NPAIR], [1, 512]]))
            sl = oTp[rh * HD:(rh + 1) * HD, :, qt * 512:(qt + 1) * 512]
            nc.vector.tensor_tensor(out=sl, in0=sl, in1=rbc, op=ALU.mult)

        # ---- D/E helpers ----
        def outproj_rows(it):
            """x2 row it = x + oTp @ wo."""
            for ct in range(2):
                ps = work_ps.tile([P, 512], F32, tag="w", name="opj")
                for hp in range(4):
                    lhs = fap(oTp[:, 0, 0:1], (2 * hp) * S + it * P,
                              [[S, 2], [1, P]])
                    rhs = fap(wo8[:, 0, 0:1], (2 * hp) * D + ct * 512,
                              [[D, 2], [1, 512]])
                    nc.tensor.matmul(ps, lhsT=lhs, rhs=rhs, start=(hp == 0),
                                     stop=zb and (hp == 3), perf_mode=DR,
                                     skip_group_check=True)
                if not zb:
                    bias_mm(ps[:, 0:256], bo8, ct * 512, 256, False, False)
                    bias_mm(ps[:, 256:512], bo8, ct * 512 + 256, 256, False, True)
                xr = xrp.tile([P, 512], BF16, tag="xr2", name="xr2")
                nc.gpsimd.dma_start(
                    xr, x[it * P:(it + 1) * P, ct * 512:(ct + 1) * 512])
                nc.gpsimd.tensor_add(out=x2[:, it, ct * 512:(ct + 1) * 512],
                                      in0=ps, in1=xr)

        def ln2_row(it):
            rstd, negms = ln_rstd(x2[:, it, :], ln2p)
            y = yln.tile([P, D], F32R, tag="y", name="yy")
            nc.vector.tensor_scalar(out=y, in0=x2[:, it, :], scalar1=rstd,
                                    scalar2=negms, op0=ALU.mult, op1=ALU.add)
            ln_transpose(it, y, y2h, slot1=True)

        def mlp1_half_iter(sh, ps_pool):
            """MLP1 for all 32 ff blocks of half sh (streams w1); yields
            after each block so callers can interleave emission."""
            w1c = None
            for ft in range(FT):
                if ft % 2 == 0:
                    w1c = wch.tile([P, DT, 2, 256], FP8, tag="w1c", name="w1c")
                    nc.sync.dma_start(w1c, wpack1[ft // 2])
                ps = ps_pool.tile([P, 512], F32, tag="w", name="m1")
                for kt in range(DT):
                    lhsA = fap(w1c[:, 0, 0, 0:1], kt * 512 + (ft % 2) * P,
                               [[256, 2], [1, P]])
                    rhsA = fap(y2h[:, 0, 0, 0:1], kt * 2 * S + sh * 512,
                               [[S, 2], [1, 512]])
                    nc.tensor.matmul(ps, lhsT=lhsA, rhs=rhsA, start=(kt == 0),
                                     stop=(kt == DT - 1), perf_mode=DR,
                                     skip_group_check=True)
                nc.gpsimd.tensor_copy(out=h_pre[:, ft, :], in_=ps)
                yield ft

        def mlp1_half(sh, ps_pool):
            for _ in mlp1_half_iter(sh, ps_pool):
                pass

        # ======== q-half 0 epoch ========
        v_proj(0)           # heads 0-7: covers PV of pairs 0-3
        attn_pair(0, 0)
        v_proj(1)           # heads 8-15
        attn_pair(1, 0)
        for p in range(2, NPAIR):
            attn_pair(p, 0)
            if p == 3:
                normalize_group(0, 0)
            q_proj(p - 2, 1)
        normalize_group(0, 1)
        if DEBUG_DUMPS:
            nc.sync.dma_start(d_y1, y1)
            nc.sync.dma_start(d_vext, v_ext)
            nc.sync.dma_start(d_kT, kT)
            nc.sync.dma_start(d_qT, qT)
        q_proj(6, 1)
        q_proj(7, 1)
        wqy_ctx.close()

        # ======== q-half 1 epoch: D/E(h0) first, MLP1(h0) interleaved ====
        mlp1_iter = mlp1_half_iter(0, work_ps)
        for p in range(NPAIR):
            attn_pair(p, 1)
            if p == 0:
                for it in range(4):
                    outproj_rows(it)
                    ln2_row(it)
            else:
                for _ in range(5):
                    next(mlp1_iter, None)
            if p == 3:
                normalize_group(1, 0)
        normalize_group(1, 1)
        att_ctx.close()

        if DEBUG_DUMPS:
            nc.sync.dma_start(d_oTp, oTp)
        # D/E half 1 rows; finish any remaining MLP1(half0) blocks
        for it in range(4, ST):
            outproj_rows(it)
            ln2_row(it)
        for _ in mlp1_iter:
            pass
        if DEBUG_DUMPS:
            nc.sync.dma_start(d_x2, x2)
            nc.sync.dma_start(d_y2h, y2h)
            nc.sync.dma_start(d_hpre, h_pre)
        mid_ctx.close()

        # ======== gelu + MLP2 epochs ========
        tail_ctx = contextlib.ExitStack()
        h1p = tail_ctx.enter_context(tc.tile_pool(name="h1p", bufs=3))
        w2p = tail_ctx.enter_context(tc.tile_pool(name="w2p", bufs=3))
        hgp = tail_ctx.enter_context(tc.tile_pool(name="hgp", bufs=3))

        def mlp2_half(sh):
            with tc.tile_pool(name="m2_ps", bufs=1, space="PSUM") as m2_ps:
                tiles = [m2_ps.tile([P, 512], F32, tag=f"m2_{i}", name=f"m2_{i}")
                         for i in range(8)]
                if not zb:
                    for i, tl in enumerate(tiles):
                        il, cth = i // 2, i % 2
                        bias_mm(tl[:, 0:256], bb2, cth * 512, 256, True, False)
                        bias_mm(tl[:, 256:512], bb2, cth * 512 + 256, 256,
                                False, False)
                for bk in range(16):
                    w2k = w2p.tile([P, 2, 2, D], FP8, tag="w2k", name="w2k")
                    nc.gpsimd.dma_start(w2k, w2kpack[bk])
                    h1t = h1p.tile([P, 2, 3, 512], FP8, tag="h1t", name="h1t")
                    hg = hgp.tile([P, 2, 512], BF16, tag="hg", name="hg")
                    # gelu (bf16): 2 ff-blocks from h_pre
                    if zb:
                        gin = fap(h_pre[:, 0, 0:1], (2 * bk) * 512, [[1, 1024]])
                        nc.scalar.activation(out=hg, in_=gin, func=AF.Gelu,
                                             bias=0.0, scale=1.0)
                    else:
                        for k2 in range(2):
                            nc.scalar.activation(
                                out=hg[:, k2, :],
                                in_=h_pre[:, 2 * bk + k2, :], func=AF.Gelu,
                                bias=b1c[:, 2 * bk + k2:2 * bk + k2 + 1],
                                scale=1.0)
                    for k2 in range(2):
                        nc.vector.tensor_copy(out=h1t[:, k2, 0, :],
                                              in_=hg[:, k2, :])
                        nc.vector.tensor_scalar(out=h1t[:, k2, 1, :],
                                                in0=h1t[:, k2, 0, :],
                                                scalar1=c32_t, scalar2=None,
                                                op0=ALU.mult)
                        nc.gpsimd.scalar_tensor_tensor(
                            out=h1t[:, k2, 2, :], in0=h1t[:, k2, 0, :],
                            scalar=-1.0, in1=hg[:, k2, :],
                            op0=ALU.mult, op1=ALU.add)
                    for i, tl in enumerate(tiles):
                        il, cth = i // 2, i % 2
                        for k2 in range(2):
                            lhsA = fap(h1t[:, 0, 0, 0:1], k2 * 1536 + il * P,
                                       [[512, 2], [1, P]])
                            rhsA = fap(w2k[:, 0, 0, 0:1], k2 * 2 * D + cth * 512,
                                       [[D, 2], [1, 512]])
                            nc.tensor.matmul(
                                tl, lhsT=lhsA, rhs=rhsA,
                                start=(zb and bk == 0 and k2 == 0),
                                stop=(bk == 15 and k2 == 1 and NO_LO),
                                perf_mode=DR, skip_group_check=True)
                        if not NO_LO:
                            lhsB = fap(h1t[:, 0, 0, 0:1], 2 * 512 + il * P,
                                       [[1536, 2], [1, P]])
                            rhsB = fap(w2k[:, 0, 0, 0:1], cth * 512,
                                       [[2 * D, 2], [1, 512]])
                            nc.tensor.matmul(tl, lhsT=lhsB, rhs=rhsB, start=False,
                                             stop=(bk == 15), perf_mode=DR,
                                             skip_group_check=True)
                for i, tl in enumerate(tiles):
                    il, cth = i // 2, i % 2
                    it = sh * 4 + il
                    ot = outp.tile([P, 512], F32, tag="fin")
                    nc.vector.tensor_add(
                        out=ot, in0=tl,
                        in1=x2[:, it, cth * 512:(cth + 1) * 512])
                    nc.sync.dma_start(
                        out=out[it * P:(it + 1) * P, cth * 512:(cth + 1) * 512],
                        in_=ot)

        mlp2_half(0)
        with tc.tile_pool(name="m1b_ps", bufs=2, space="PSUM") as m1b_ps:
            mlp1_half(1, m1b_ps)
        mlp2_half(1)
        tail_ctx.close()

    nc.compile()
    return nc


_NC_CACHE = {}


def _get_nc(zb=True):
    if zb not in _NC_CACHE:
        _NC_CACHE[zb] = build_program(zb)
    return _NC_CACHE[zb]


def _q8(a):
    return a.astype(ml_dtypes.float8_e4m3)


def _prep_weights(inputs):
    f32 = lambda k: np.asarray(inputs[k], np.float32)
    ln1_g, ln1_b = f32("ln1_g"), f32("ln1_b")
    ln2_g, ln2_b = f32("ln2_g"), f32("ln2_b")
    w_qkv, w_out, b_out = f32("w_qkv"), f32("w_out"), f32("b_out")
    w1, b1, w2, b2 = f32("w1"), f32("b1"), f32("w2"), f32("b2")

    wqkv_g = w_qkv * ln1_g[:, None]
    wqkv8 = np.ascontiguousarray(
        _q8(wqkv_g).reshape(DT, P, 3 * D).transpose(1, 0, 2))
    bias_qkv = ln1_b @ w_qkv
    qk_bias = np.empty((P, 2, NPAIR), np.float32)
    for pp in range(NPAIR):
        qk_bias[:, 0, pp] = bias_qkv[D + pp * P:D + (pp + 1) * P]
        qk_bias[:, 1, pp] = bias_qkv[2 * D + pp * P:2 * D + (pp + 1) * P]
    vbias8 = _q8(bias_qkv[:D]).reshape(1, D)
    ones_aux = np.zeros((1, 2 * P), np.float32)
    ones_aux[0, :P] = 1.0

    # out-proj, head-pair stacked: wop8[p, hp, :] = w_out[row], where
    # partition p<64 -> head hp feat p ; p>=64 -> head hp+8 feat p-64.
    wr = _q8(w_out).reshape(H, HD, D)
    wop8 = np.ascontiguousarray(np.concatenate(
        [wr[0:8].transpose(1, 0, 2), wr[8:16].transpose(1, 0, 2)],
        axis=0))  # [128, 8, D]

    w1_g = w1 * ln2_g[:, None]
    w1h = _q8(w1_g)
    w1l = _q8(32.0 * (w1_g - w1h.astype(np.float32)))
    # [fc 16, P part, DT kt, 2 (hi, 32*lo), 256] chunk-contiguous
    wpack1 = np.ascontiguousarray(np.stack(
        [w1h.reshape(DT, P, 16, 256).transpose(2, 1, 0, 3),
         w1l.reshape(DT, P, 16, 256).transpose(2, 1, 0, 3)], axis=3))
    bias1 = ln2_b @ w1 + b1
    b1_col = np.ascontiguousarray(bias1.reshape(FT, P).T)

    w2h = _q8(w2).reshape(FT, P, D)
    w2l = _q8(32.0 * (w2 - _q8(w2).astype(np.float32))).reshape(FT, P, D)
    # [bk 16, P, kt2 2, slot 2, D] k-pair-chunk contiguous
    w2kpack = np.ascontiguousarray(
        np.stack([w2h, w2l], axis=1).reshape(16, 2, 2, P, D)
        .transpose(0, 3, 1, 2, 4))

    return {
        "wqkv8": wqkv8, "qk_bias": qk_bias, "vbias8": vbias8,
        "ones_aux8": _q8(ones_aux), "wop8": wop8,
        "bout8": _q8(b_out).reshape(1, D), "b1_col": b1_col,
        "wpack1": wpack1, "w2kpack": w2kpack,
        "b2_8": _q8(b2).reshape(1, D),
    }, (not np.any(bias_qkv) and not np.any(b_out) and not np.any(bias1)
        and not np.any(b2))


WEIGHT_NAMES = ["wqkv8", "qk_bias", "vbias8", "ones_aux8", "wop8", "bout8",
                "b1_col", "wpack1", "w2kpack", "b2_8"]


def kernel(**inputs) -> np.ndarray:
    x = np.asarray(inputs["x"], dtype=np.float32).astype(ml_dtypes.bfloat16)
    B = x.shape[0]
    weights, zb = _prep_weights(inputs)
    nc = _get_nc(zb)
    in_maps = [{"x": np.ascontiguousarray(x[b]), **weights} for b in range(B)]
    res = bass_utils.run_bass_kernel_spmd(nc, in_maps, core_ids=list(range(B)))
    return np.stack([res.results[b]["out"] for b in range(B)], axis=0)


# revision 40
# speedup vs baseline: 1.0053x; 1.0046x over previous
"""Trainium2 Bass kernel for a dense transformer block (pre-LN, MHA + MLP).

Data-parallel over batch: 8 batch elements, one per NeuronCore; weights
replicated, no collectives.

All GEMMs are fp8e4 (e4m3) DoubleRow matmuls (0.5 cycles/row).  Precision
(CPU-validated max scale-rel err ~1.6e-2 vs the 2e-2 gate):
  - attention (QKV proj, scores, P@V, out proj): plain fp8 both operands.
  - MLP1 2-term: (w1h, 32*w1l) x (y2h, y2h/32)  [drops W@y2_lo].
  - MLP2 3-term: (w2h, 32*w2l) x (h8, h8/32) + unscaled hl x w2h.
  - softmax: p = exp(s/8 - log 16) fp8; denominators from an appended
    0.25-scaled ones column in V; reciprocal broadcast via SBUF->SBUF DMA
    and folded into the PSUM->SBUF drain of o^T.
  - LN rstd = exp(-0.5*ln(var+eps)) so softmax-exp, ln and identity share
    one ACT table; gelus batched in a second table epoch (2 loads total).

Schedule: LN1 -> K/Q0/V proj -> [attention q-half 0 | exp epoch] ->
[q-half 1 | exp, with out-proj/LN2/MLP1 of half 0 filling the PE] ->
out-proj/LN2(h1) -> [gelu+MLP2(h0) pipelined per k-block] -> MLP1(h1) ->
[gelu+MLP2(h1)].  MLP2 streams w2 k-major so gelu output feeds matmuls
incrementally; MLP1 stages pre-gelu in bf16 (h_pre) so ACT table epochs
stay contiguous.
"""
import contextlib
import os
import sys

import numpy as np
import ml_dtypes

DEBUG_DUMPS = bool(os.environ.get("BASSDBG"))

sys.path.insert(0, "/opt/trn_rl_repo")

import concourse.bass as bass
import concourse.mybir as mybir
import concourse.tile as tile
from concourse import bacc, bass_utils
from concourse.masks import make_identity

F32 = mybir.dt.float32
F32R = mybir.dt.float32r
BF16 = mybir.dt.bfloat16
FP8 = mybir.dt.float8e4
AF = mybir.ActivationFunctionType
ALU = mybir.AluOpType
DR = mybir.MatmulPerfMode.DoubleRow

P = 128
S = 1024
D = 1024
H = 16
HD = 64
FF = 4096
ST = S // P   # 8
DT = D // P   # 8
FT = FF // P  # 32
NPAIR = H // 2
EPS = 1e-5
NLOG16 = -2.7725887
NO_LO = bool(os.environ.get("NOLO"))


def fap(base, off, dims):
    """AP with base's partition dim, extra element offset, custom free dims."""
    return bass.AP(tensor=base.tensor, offset=base.offset + off,
                   ap=[list(base.ap[0])] + [list(d) for d in dims])


def build_program(zb: bool):
    """zb=True builds the fast path for all-zero biases/unit-free foldings
    (the actual setup_inputs case); zb=False adds the bias instructions."""
    nc = bacc.Bacc("TRN2", target_bir_lowering=False, debug=False)

    x = nc.dram_tensor("x", [S, D], BF16, kind="ExternalInput").ap()
    wqkv8 = nc.dram_tensor("wqkv8", [P, DT, 3 * D], FP8, kind="ExternalInput").ap()
    wop8 = nc.dram_tensor("wop8", [P, NPAIR, D], FP8, kind="ExternalInput").ap()
    wpack1 = nc.dram_tensor("wpack1", [16, P, DT, 2, 256], FP8,
                            kind="ExternalInput").ap()
    w2kpack = nc.dram_tensor("w2kpack", [16, P, 2, 2, D], FP8,
                             kind="ExternalInput").ap()
    qk_bias = nc.dram_tensor("qk_bias", [P, 2, NPAIR], F32, kind="ExternalInput").ap()
    vbias8 = nc.dram_tensor("vbias8", [1, D], FP8, kind="ExternalInput").ap()
    ones_aux8 = nc.dram_tensor("ones_aux8", [1, 2 * P], FP8, kind="ExternalInput").ap()
    bout8 = nc.dram_tensor("bout8", [1, D], FP8, kind="ExternalInput").ap()
    b1_col = nc.dram_tensor("b1_col", [P, FT], F32, kind="ExternalInput").ap()
    b2_8 = nc.dram_tensor("b2_8", [1, D], FP8, kind="ExternalInput").ap()
    out = nc.dram_tensor("out", [S, D], F32, kind="ExternalOutput").ap()
    if DEBUG_DUMPS:
        d_y1 = nc.dram_tensor("d_y1", [P, DT, S], FP8, kind="ExternalOutput").ap()
        d_vext = nc.dram_tensor("d_vext", [P, ST, H, HD + 1], FP8,
                                kind="ExternalOutput").ap()
        d_qT = nc.dram_tensor("d_qT", [P, NPAIR, 2, 512], FP8,
                              kind="ExternalOutput").ap()
        d_kT = nc.dram_tensor("d_kT", [P, NPAIR, S], FP8,
                              kind="ExternalOutput").ap()
        d_oTp = nc.dram_tensor("d_oTp", [P, NPAIR, S], FP8,
                               kind="ExternalOutput").ap()
        d_x2 = nc.dram_tensor("d_x2", [P, ST, D], BF16, kind="ExternalOutput").ap()
        d_y2h = nc.dram_tensor("d_y2h", [P, DT, 2, S], FP8,
                               kind="ExternalOutput").ap()
        d_hpre = nc.dram_tensor("d_hpre", [P, FT, 512], BF16,
                                kind="ExternalOutput").ap()

    with tile.TileContext(nc) as tc, contextlib.ExitStack() as ctx:
        singles = ctx.enter_context(tc.tile_pool(name="singles", bufs=1))
        big = ctx.enter_context(tc.tile_pool(name="big", bufs=1))
        outp = ctx.enter_context(tc.tile_pool(name="outp", bufs=2))
        dram = ctx.enter_context(tc.tile_pool(name="dram", bufs=1, space="DRAM"))

        # ---- constants ----
        ident = singles.tile([P, P], F32)
        make_identity(nc, ident)
        identr = singles.tile([P, P], F32R)
        nc.vector.tensor_copy(out=identr, in_=ident)
        eps_t = singles.tile([P, 1], F32)
        nc.vector.memset(eps_t, EPS)
        nbias_t = singles.tile([P, 1], F32)
        nc.vector.memset(nbias_t, NLOG16)
        c32_t = singles.tile([P, 1], F32)
        nc.vector.memset(c32_t, 1.0 / 32.0)
        cq_t = singles.tile([P, 1], F32)
        nc.vector.memset(cq_t, 0.25)
        cm1_t = singles.tile([P, 1], F32)
        nc.vector.memset(cm1_t, -1.0)
        if not zb:
            onesz = singles.tile([1, 2, P], FP8)
            nc.sync.dma_start(onesz, ones_aux8.rearrange("o (t p) -> o t p", t=2))
            vb8 = singles.tile([1, D], FP8)
            nc.sync.dma_start(vb8, vbias8)
            bo8 = singles.tile([1, D], FP8)
            nc.sync.dma_start(bo8, bout8)
            bb2 = singles.tile([1, D], FP8)
            nc.sync.dma_start(bb2, b2_8)
            qkb = singles.tile([P, 2, NPAIR], F32)
            nc.sync.dma_start(qkb, qk_bias)
        b1c = singles.tile([P, FT], F32)
        nc.sync.dma_start(b1c, b1_col)

        def bias_mm(ps_ap, row8, c0, n, start, stop):
            rhs = fap(row8[0:1], c0, [[0, 2], [1, n]])
            nc.tensor.matmul(ps_ap, lhsT=onesz, rhs=rhs, start=start, stop=stop,
                             perf_mode=DR, skip_group_check=True)

        # ---- persistent tiles (never released) ----
        kT = big.tile([P, NPAIR, S], FP8, tag="kT")          # 8KB
        qT = big.tile([P, NPAIR, 2, 512], FP8, tag="qT")     # 8KB (slot1 zero)
        v_ext = big.tile([P, ST, H, HD + 1], FP8, tag="vx")  # 8.5KB
        oTp = big.tile([P, NPAIR, S], FP8, tag="oTp")        # 8KB
        y2h = big.tile([P, DT, 2, S], FP8, tag="y2h")        # 16KB
        x2 = big.tile([P, ST, D], BF16, tag="x2")            # 16KB
        wo8 = big.tile([P, NPAIR, D], FP8, tag="wo8")        # 8KB
        h_pre = big.tile([P, FT, 512], BF16, tag="hpre")     # 32KB (per half)
        wch = ctx.enter_context(tc.tile_pool(name="wch", bufs=2))

        # mid-lifetime pools: [start, end of D/E half1]
        mid_ctx = contextlib.ExitStack()
        work_ps = mid_ctx.enter_context(
            tc.tile_pool(name="work_ps", bufs=2, space="PSUM"))
        yln = mid_ctx.enter_context(tc.tile_pool(name="yln", bufs=2))
        ln2p = mid_ctx.enter_context(tc.tile_pool(name="ln2p", bufs=4))
        xrp = mid_ctx.enter_context(tc.tile_pool(name="xrp", bufs=2))
        # attention pools
        att_ctx = contextlib.ExitStack()
        ssc_ps = att_ctx.enter_context(
            tc.tile_pool(name="ssc_ps", bufs=2, space="PSUM"))
        ot_ps = att_ctx.enter_context(
            tc.tile_pool(name="ot_ps", bufs=2, space="PSUM"))
        ptp = att_ctx.enter_context(tc.tile_pool(name="ptp", bufs=8))
        rdp = att_ctx.enter_context(tc.tile_pool(name="rdp", bufs=3))
        rbp = att_ctx.enter_context(tc.tile_pool(name="rbp", bufs=1))
        # qkv weights + y1 live only through the projections
        wqy_ctx = contextlib.ExitStack()
        wqy = wqy_ctx.enter_context(tc.tile_pool(name="wqy", bufs=1))
        wq8 = wqy.tile([P, DT, 3 * D], FP8, tag="wq8")       # 24KB
        y1 = wqy.tile([P, DT, S], FP8, tag="y1")             # 8KB
        lnp_ctx = contextlib.ExitStack()
        lnp = lnp_ctx.enter_context(tc.tile_pool(name="lnp", bufs=4))

        nc.gpsimd.memset(qT[:, :, 1, :], 0.0)
        nc.gpsimd.memset(v_ext[:, :, :, HD:HD + 1], 0.25)

        # ---------------- LN helpers ----------------
        def ln_rstd(x_row, ln):
            """bn stats + rstd = exp(-0.5 ln(var+eps)); returns (rstd, negms)."""
            stats = ln.tile([P, 2, 6], F32, tag="stats")
            xg = x_row.rearrange("p (n f) -> p n f", f=512)
            for g in range(2):
                nc.vector.bn_stats(out=stats[:, g, :], in_=xg[:, g, :])
            mv = ln.tile([P, 2], F32, tag="mv")
            nc.vector.bn_aggr(out=mv, in_=stats)
            rstd = ln.tile([P, 1], F32, tag="rstd")
            nc.scalar.activation(out=rstd, in_=mv[:, 1:2], func=AF.Ln,
                                 bias=eps_t, scale=1.0)
            nc.scalar.activation(out=rstd, in_=rstd, func=AF.Exp,
                                 bias=0.0, scale=-0.5)
            negms = ln.tile([P, 1], F32, tag="negms")
            nc.vector.tensor_scalar(out=negms, in0=mv[:, 0:1], scalar1=rstd,
                                    scalar2=cm1_t, op0=ALU.mult, op1=ALU.mult)
            return rstd, negms

        def ln_transpose(st, y, yh, slot1):
            """transpose y (f32r row) into feature-major fp8 yh; slot1 -> /32."""
            for dg in range(2):
                ps = work_ps.tile([P, 512], F32, tag="w", name="tp_ps")
                for j in range(4):
                    dt = dg * 4 + j
                    nc.tensor.transpose(
                        ps[:, j * P:(j + 1) * P].bitcast(F32R),
                        y[:, dt * P:(dt + 1) * P], identr)
                if slot1:
                    oap = fap(yh[:, 0, 0, 0:1], (dg * 4) * 2 * S + st * P,
                              [[2 * S, 4], [1, P]])
                    o32 = fap(yh[:, 0, 0, 0:1], (dg * 4) * 2 * S + S + st * P,
                              [[2 * S, 4], [1, P]])
                    nc.vector.tensor_copy(out=oap, in_=ps)
                    nc.vector.tensor_scalar(out=o32, in0=ps, scalar1=c32_t,
                                            scalar2=None, op0=ALU.mult)
                else:
                    oap = fap(yh[:, 0, 0:1], (dg * 4) * S + st * P,
                              [[S, 4], [1, P]])
                    nc.vector.tensor_copy(out=oap, in_=ps)

        # ---------------- Phase A: LN1 ----------------
        for st in range(ST):
            xr = xrp.tile([P, D], BF16, tag="xln", name="xln")
            nc.gpsimd.dma_start(xr, x[st * P:(st + 1) * P, :])
            if st == 1:
                nc.sync.dma_start(wq8, wqkv8)
            if st == 4:
                nc.sync.dma_start(wo8, wop8)
            rstd, negms = ln_rstd(xr, lnp)
            y = yln.tile([P, D], F32R, tag="y", name="yy")
            nc.scalar.activation(out=y, in_=xr, func=AF.Identity,
                                 scale=rstd, bias=negms)
            ln_transpose(st, y, y1, slot1=False)
        lnp_ctx.close()

        # ---------------- projections ----------------
        def proj_feat(col0, dst_part, dst_ap, bias_col):
            ps = work_ps.tile([P, 512], F32, tag="w", name="proj")
            for kp in range(4):
                lhs = fap(wq8[:, 0, 0:1], (2 * kp) * 3 * D + col0,
                          [[3 * D, 2], [1, P]])
                rhs = fap(y1[:, 0, 0:1], (2 * kp) * S + dst_part * 512,
                          [[S, 2], [1, 512]])
                nc.tensor.matmul(ps, lhsT=lhs, rhs=rhs, start=(kp == 0),
                                 stop=(kp == 3), perf_mode=DR,
                                 skip_group_check=True)
            if zb:
                nc.vector.tensor_copy(out=dst_ap, in_=ps)
            else:
                nc.vector.tensor_scalar(out=dst_ap, in0=ps, scalar1=bias_col,
                                        scalar2=None, op0=ALU.add)

        def k_proj(pr):
            for sh in range(2):
                dst = kT[:, pr, sh * 512:(sh + 1) * 512]
                proj_feat(2 * D + pr * P, sh, dst,
                          None if zb else qkb[:, 1, pr:pr + 1])

        def q_proj(pr, qt):
            proj_feat(D + pr * P, qt, qT[:, pr, 0, :],
                      None if zb else qkb[:, 0, pr:pr + 1])

        def v_proj(vc):
            for it in range(ST):
                ps = work_ps.tile([P, 512], F32, tag="w", name="vproj")
                for kp in range(4):
                    lhs = fap(y1[:, 0, 0:1], (2 * kp) * S + it * P,
                              [[S, 2], [1, P]])
                    rhs = fap(wq8[:, 0, 0:1], (2 * kp) * 3 * D + vc * 512,
                              [[3 * D, 2], [1, 512]])
                    nc.tensor.matmul(ps, lhsT=lhs, rhs=rhs, start=(kp == 0),
                                     stop=zb and (kp == 3), perf_mode=DR,
                                     skip_group_check=True)
                if not zb:
                    bias_mm(ps[:, 0:256], vb8, vc * 512, 256, False, False)
                    bias_mm(ps[:, 256:512], vb8, vc * 512 + 256, 256, False, True)
                oap = fap(v_ext[:, 0, 0, 0:1], it * H * (HD + 1) + vc * 8 * (HD + 1),
                          [[HD + 1, 8], [1, HD]])
                nc.vector.tensor_scalar(out=oap,
                                        in0=ps.rearrange("p (h c) -> p h c", c=HD),
                                        scalar1=cq_t, scalar2=None, op0=ALU.mult)

        for pr in range(NPAIR):
            k_proj(pr)
        for pr in range(NPAIR):
            q_proj(pr, 0)

        # ---------------- attention ----------------
        recip_dram = dram.tile([H, 2, 512], F32)

        def attn_pair(p, qt):
            """scores/exp/PV/recip/normalized-drain for head pair p, half qt."""
            pts = []
            for e in range(2):
                for jh in range(4):      # key-tile groups of 2
                    ssc = ssc_ps.tile([P, 2, 512], F32, tag="ssc")
                    for jj in range(2):
                        jt = jh * 2 + jj
                        lhs = fap(kT[e * HD:(e + 1) * HD, 0, 0:1],
                                  p * S + jt * P, [[0, 2], [1, P]])
                        rhs = fap(qT[e * HD:(e + 1) * HD, 0, 0, 0:1], p * 1024,
                                  [[512, 2], [1, 512]])
                        nc.tensor.matmul(ssc[:, jj, :], lhsT=lhs, rhs=rhs,
                                         start=True, stop=True,
                                         perf_mode=DR, skip_group_check=True)
                    pt = ptp.tile([P, 2, 512], FP8, tag="pT")
                    nc.scalar.activation(out=pt, in_=ssc, func=AF.Exp,
                                         scale=0.125, bias=nbias_t)
                    pts.append(pt)
            for e in range(2):
                ot = ot_ps.tile([HD + 1, 512], F32, tag="ot")
                h = 2 * p + e
                for jc in range(4):
                    lhs = fap(v_ext[:, 0, 0, 0:1],
                              (jc * 2) * H * (HD + 1) + h * (HD + 1),
                              [[H * (HD + 1), 2], [1, HD + 1]])
                    rhs = fap(pts[4 * e + jc][:, 0, 0:1], 0,
                              [[512, 2], [1, 512]])
                    nc.tensor.matmul(ot, lhsT=lhs, rhs=rhs, start=(jc == 0),
                                     stop=(jc == 3), perf_mode=DR,
                                     skip_group_check=True)
                rd = rdp.tile([1, 512], F32, tag="rd")
                nc.vector.reciprocal(out=rd, in_=ot[HD:HD + 1, :])
                nc.sync.dma_start(
                    out=recip_dram.bitcast(F32)[h:h + 1, qt, :], in_=rd)
                rh, hsl = h // 8, h % 8
                dst = oTp[rh * HD:(rh + 1) * HD, hsl, qt * 512:(qt + 1) * 512]
                nc.vector.tensor_copy(out=dst, in_=ot[0:HD, :])

        def normalize_group(qt, rh):
            """oTp[rh half, :, qt half] *= recip broadcast (8 heads batched)."""
            rbc = rbp.tile([HD, NPAIR, 512], F32, tag="rbc", name="rbc")
            nc.sync.dma_start(out=rbc, in_=bass.AP(
                tensor=recip_dram.tensor,
                offset=recip_dram.offset + (rh * 8 * 2 + qt) * 512,
                ap=[[0, HD], [2 * 512, NPAIR], [1, 512]]))
            sl = oTp[rh * HD:(rh + 1) * HD, :, qt * 512:(qt + 1) * 512]
            nc.vector.tensor_tensor(out=sl, in0=sl, in1=rbc, op=ALU.mult)

        # ---- D/E helpers ----
        def outproj_rows(it):
            """x2 row it = x + oTp @ wo."""
            for ct in range(2):
                ps = work_ps.tile([P, 512], F32, tag="w", name="opj")
                for hp in range(4):
                    lhs = fap(oTp[:, 0, 0:1], (2 * hp) * S + it * P,
                              [[S, 2], [1, P]])
                    rhs = fap(wo8[:, 0, 0:1], (2 * hp) * D + ct * 512,
                              [[D, 2], [1, 512]])
                    nc.tensor.matmul(ps, lhsT=lhs, rhs=rhs, start=(hp == 0),
                                     stop=zb and (hp == 3), perf_mode=DR,
                                     skip_group_check=True)
                if not zb:
                    bias_mm(ps[:, 0:256], bo8, ct * 512, 256, False, False)
                    bias_mm(ps[:, 256:512], bo8, ct * 512 + 256, 256, False, True)
                xr = xrp.tile([P, 512], BF16, tag="xr2", name="xr2")
                nc.gpsimd.dma_start(
                    xr, x[it * P:(it + 1) * P, ct * 512:(ct + 1) * 512])
                nc.gpsimd.tensor_add(out=x2[:, it, ct * 512:(ct + 1) * 512],
                                      in0=ps, in1=xr)

        def ln2_row(it):
            rstd, negms = ln_rstd(x2[:, it, :], ln2p)
            y = yln.tile([P, D], F32R, tag="y", name="yy")
            nc.vector.tensor_scalar(out=y, in0=x2[:, it, :], scalar1=rstd,
                                    scalar2=negms, op0=ALU.mult, op1=ALU.add)
            ln_transpose(it, y, y2h, slot1=True)

        def mlp1_half_iter(sh, ps_pool):
            """MLP1 for all 32 ff blocks of half sh (streams w1); yields
            after each block so callers can interleave emission."""
            w1c = None
            for ft in range(FT):
                if ft % 2 == 0:
                    w1c = wch.tile([P, DT, 2, 256], FP8, tag="w1c", name="w1c")
                    nc.sync.dma_start(w1c, wpack1[ft // 2])
                ps = ps_pool.tile([P, 512], F32, tag="w", name="m1")
                for kt in range(DT):
                    lhsA = fap(w1c[:, 0, 0, 0:1], kt * 512 + (ft % 2) * P,
                               [[256, 2], [1, P]])
                    rhsA = fap(y2h[:, 0, 0, 0:1], kt * 2 * S + sh * 512,
                               [[S, 2], [1, 512]])
                    nc.tensor.matmul(ps, lhsT=lhsA, rhs=rhsA, start=(kt == 0),
                                     stop=(kt == DT - 1), perf_mode=DR,
                                     skip_group_check=True)
                nc.gpsimd.tensor_copy(out=h_pre[:, ft, :], in_=ps)
                yield ft

        def mlp1_half(sh, ps_pool):
            for _ in mlp1_half_iter(sh, ps_pool):
                pass

        # ======== q-half 0 epoch ========
        v_proj(0)           # heads 0-7: covers PV of pairs 0-3
        attn_pair(0, 0)
        v_proj(1)           # heads 8-15
        attn_pair(1, 0)
        for p in range(2, NPAIR):
            attn_pair(p, 0)
            if p == 3:
                normalize_group(0, 0)
            q_proj(p - 2, 1)
        normalize_group(0, 1)
        if DEBUG_DUMPS:
            nc.sync.dma_start(d_y1, y1)
            nc.sync.dma_start(d_vext, v_ext)
            nc.sync.dma_start(d_kT, kT)
            nc.sync.dma_start(d_qT, qT)
        q_proj(6, 1)
        q_proj(7, 1)
        wqy_ctx.close()

        # ======== q-half 1 epoch: D/E(h0) first, MLP1(h0) interleaved ====
        mlp1_iter = mlp1_half_iter(0, work_ps)
        for p in range(NPAIR):
            attn_pair(p, 1)
            if p == 0:
                for it in range(4):
                    outproj_rows(it)
                    ln2_row(it)
            else:
                for _ in range(5):
                    next(mlp1_iter, None)
            if p == 3:
                normalize_group(1, 0)
        normalize_group(1, 1)
        att_ctx.close()

        if DEBUG_DUMPS:
            nc.sync.dma_start(d_oTp, oTp)
        # D/E half 1 rows; finish any remaining MLP1(half0) blocks
        for it in range(4, ST):
            outproj_rows(it)
            ln2_row(it)
        for _ in mlp1_iter:
            pass
        if DEBUG_DUMPS:
            nc.sync.dma_start(d_x2, x2)
            nc.sync.dma_start(d_y2h, y2h)
            nc.sync.dma_start(d_hpre, h_pre)
        mid_ctx.close()

        # ======== gelu + MLP2 epochs ========
        tail_ctx = contextlib.ExitStack()
        h1p = tail_ctx.enter_context(tc.tile_pool(name="h1p", bufs=3))
        w2p = tail_ctx.enter_context(tc.tile_pool(name="w2p", bufs=3))
        hgp = tail_ctx.enter_context(tc.tile_pool(name="hgp", bufs=3))

        def mlp2_half(sh):
            with tc.tile_pool(name="m2_ps", bufs=1, space="PSUM") as m2_ps:
                tiles = [m2_ps.tile([P, 512], F32, tag=f"m2_{i}", name=f"m2_{i}")
                         for i in range(8)]
                if not zb:
                    for i, tl in enumerate(tiles):
                        il, cth = i // 2, i % 2
                        bias_mm(tl[:, 0:256], bb2, cth * 512, 256, True, False)
                        bias_mm(tl[:, 256:512], bb2, cth * 512 + 256, 256,
                                False, False)
                for bk in range(16):
                    w2k = w2p.tile([P, 2, 2, D], FP8, tag="w2k", name="w2k")
                    nc.gpsimd.dma_start(w2k, w2kpack[bk])
                    h1t = h1p.tile([P, 2, 3, 512], FP8, tag="h1t", name="h1t")
                    hg = hgp.tile([P, 2, 512], BF16, tag="hg", name="hg")
                    # gelu (bf16): 2 ff-blocks from h_pre
                    if zb:
                        gin = fap(h_pre[:, 0, 0:1], (2 * bk) * 512, [[1, 1024]])
                        nc.scalar.activation(out=hg, in_=gin, func=AF.Gelu,
                                             bias=0.0, scale=1.0)
                    else:
                        for k2 in range(2):
                            nc.scalar.activation(
                                out=hg[:, k2, :],
                                in_=h_pre[:, 2 * bk + k2, :], func=AF.Gelu,
                                bias=b1c[:, 2 * bk + k2:2 * bk + k2 + 1],
                                scale=1.0)
                    for k2 in range(2):
                        nc.vector.tensor_copy(out=h1t[:, k2, 0, :],
                                              in_=hg[:, k2, :])
                        nc.vector.tensor_scalar(out=h1t[:, k2, 1, :],
                                                in0=h1t[:, k2, 0, :],
                                                scalar1=c32_t, scalar2=None,
                                                op0=ALU.mult)
                        nc.vector.scalar_tensor_tensor(
                            out=h1t[:, k2, 2, :], in0=h1t[:, k2, 0, :],
                            scalar=-1.0, in1=hg[:, k2, :],
                            op0=ALU.mult, op1=ALU.add)
                    for i, tl in enumerate(tiles):
                        il, cth = i // 2, i % 2
                        for k2 in range(2):
                            lhsA = fap(h1t[:, 0, 0, 0:1], k2 * 1536 + il * P,
                                       [[512, 2], [1, P]])
                            rhsA = fap(w2k[:, 0, 0, 0:1], k2 * 2 * D + cth * 512,
                                       [[D, 2], [1, 512]])
                            nc.tensor.matmul(
                                tl, lhsT=lhsA, rhs=rhsA,
                                start=(zb and bk == 0 and k2 == 0),
                                stop=(bk == 15 and k2 == 1 and NO_LO),
                                perf_mode=DR, skip_group_check=True)
                        if not NO_LO:
                            lhsB = fap(h1t[:, 0, 0, 0:1], 2 * 512 + il * P,
                                       [[1536, 2], [1, P]])
                            rhsB = fap(w2k[:, 0, 0, 0:1], cth * 512,
                                       [[2 * D, 2], [1, 512]])
                            nc.tensor.matmul(tl, lhsT=lhsB, rhs=rhsB, start=False,
                                             stop=(bk == 15), perf_mode=DR,
                                             skip_group_check=True)
                for i, tl in enumerate(tiles):
                    il, cth = i // 2, i % 2
                    it = sh * 4 + il
                    ot = outp.tile([P, 512], F32, tag="fin")
                    nc.vector.tensor_add(
                        out=ot, in0=tl,
                        in1=x2[:, it, cth * 512:(cth + 1) * 512])
                    nc.sync.dma_start(
                        out=out[it * P:(it + 1) * P, cth * 512:(cth + 1) * 512],
                        in_=ot)

        mlp2_half(0)
        with tc.tile_pool(name="m1b_ps", bufs=2, space="PSUM") as m1b_ps:
            mlp1_half(1, m1b_ps)
        mlp2_half(1)
        tail_ctx.close()

    nc.compile()
    return nc


_NC_CACHE = {}


def _get_nc(zb=True):
    if zb not in _NC_CACHE:
        _NC_CACHE[zb] = build_program(zb)
    return _NC_CACHE[zb]


def _q8(a):
    return a.astype(ml_dtypes.float8_e4m3)


def _prep_weights(inputs):
    f32 = lambda k: np.asarray(inputs[k], np.float32)
    ln1_g, ln1_b = f32("ln1_g"), f32("ln1_b")
    ln2_g, ln2_b = f32("ln2_g"), f32("ln2_b")
    w_qkv, w_out, b_out = f32("w_qkv"), f32("w_out"), f32("b_out")
    w1, b1, w2, b2 = f32("w1"), f32("b1"), f32("w2"), f32("b2")

    wqkv_g = w_qkv * ln1_g[:, None]
    wqkv8 = np.ascontiguousarray(
        _q8(wqkv_g).reshape(DT, P, 3 * D).transpose(1, 0, 2))
    bias_qkv = ln1_b @ w_qkv
    qk_bias = np.empty((P, 2, NPAIR), np.float32)
    for pp in range(NPAIR):
        qk_bias[:, 0, pp] = bias_qkv[D + pp * P:D + (pp + 1) * P]
        qk_bias[:, 1, pp] = bias_qkv[2 * D + pp * P:2 * D + (pp + 1) * P]
    vbias8 = _q8(bias_qkv[:D]).reshape(1, D)
    ones_aux = np.zeros((1, 2 * P), np.float32)
    ones_aux[0, :P] = 1.0

    # out-proj, head-pair stacked: wop8[p, hp, :] = w_out[row], where
    # partition p<64 -> head hp feat p ; p>=64 -> head hp+8 feat p-64.
    wr = _q8(w_out).reshape(H, HD, D)
    wop8 = np.ascontiguousarray(np.concatenate(
        [wr[0:8].transpose(1, 0, 2), wr[8:16].transpose(1, 0, 2)],
        axis=0))  # [128, 8, D]

    w1_g = w1 * ln2_g[:, None]
    w1h = _q8(w1_g)
    w1l = _q8(32.0 * (w1_g - w1h.astype(np.float32)))
    # [fc 16, P part, DT kt, 2 (hi, 32*lo), 256] chunk-contiguous
    wpack1 = np.ascontiguousarray(np.stack(
        [w1h.reshape(DT, P, 16, 256).transpose(2, 1, 0, 3),
         w1l.reshape(DT, P, 16, 256).transpose(2, 1, 0, 3)], axis=3))
    bias1 = ln2_b @ w1 + b1
    b1_col = np.ascontiguousarray(bias1.reshape(FT, P).T)

    w2h = _q8(w2).reshape(FT, P, D)
    w2l = _q8(32.0 * (w2 - _q8(w2).astype(np.float32))).reshape(FT, P, D)
    # [bk 16, P, kt2 2, slot 2, D] k-pair-chunk contiguous
    w2kpack = np.ascontiguousarray(
        np.stack([w2h, w2l], axis=1).reshape(16, 2, 2, P, D)
        .transpose(0, 3, 1, 2, 4))

    return {
        "wqkv8": wqkv8, "qk_bias": qk_bias, "vbias8": vbias8,
        "ones_aux8": _q8(ones_aux), "wop8": wop8,
        "bout8": _q8(b_out).reshape(1, D), "b1_col": b1_col,
        "wpack1": wpack1, "w2kpack": w2kpack,
        "b2_8": _q8(b2).reshape(1, D),
    }, (not np.any(bias_qkv) and not np.any(b_out) and not np.any(bias1)
        and not np.any(b2))


WEIGHT_NAMES = ["wqkv8", "qk_bias", "vbias8", "ones_aux8", "wop8", "bout8",
                "b1_col", "wpack1", "w2kpack", "b2_8"]


def kernel(**inputs) -> np.ndarray:
    x = np.asarray(inputs["x"], dtype=np.float32).astype(ml_dtypes.bfloat16)
    B = x.shape[0]
    weights, zb = _prep_weights(inputs)
    nc = _get_nc(zb)
    in_maps = [{"x": np.ascontiguousarray(x[b]), **weights} for b in range(B)]
    res = bass_utils.run_bass_kernel_spmd(nc, in_maps, core_ids=list(range(B)))
    return np.stack([res.results[b]["out"] for b in range(B)], axis=0)


# revision 41
# speedup vs baseline: 1.0649x; 1.0592x over previous
"""Trainium2 Bass kernel for a dense transformer block (pre-LN, MHA + MLP).

Data-parallel over batch: 8 batch elements, one per NeuronCore; weights
replicated, no collectives.

All GEMMs run as fp8e4 (e4m3) DoubleRow matmuls (0.5 cycles/row vs 1.0 for
fp32r): one instruction contracts 2 k-tiles (up to 256).  Accuracy plan
(CPU-validated worst-case scale-rel err ~1.1e-2 vs the 2e-2 gate):
  - attention (QKV proj, scores, P@V, out proj): plain fp8 both operands.
  - MLP1/MLP2: 3-term hi/lo compensation: W = Wh+Wl (host-packed fp8 pair),
    activations a = ah+al (fp8 pair, residual computed on-device):
    W@a ~= (Wh+Wl)@ah + Wh@al  (drops only Wl@al ~ 2^-9).
  - LN gammas folded into the following weight matrices host-side; betas
    folded into bias rows (per-partition adds for q/k; fp8 ones-row matmul
    instructions for v / b_out / b2 -- exact for the zero biases actually
    used, 1-ulp-of-fp8 otherwise).
  - softmax: p = exp(s/8 - log 16) in fp8 (max ~25 < 240), denominators from
    an appended ones column in V so the normalizer matches quantized p.
Engine placement: exp/gelu on ScalarE; casts/adds/bn on DVE; oT normalize
mults on Pool; DMA triggers on SP (hwdge) and Pool (swdge).
"""
import contextlib
import os
import sys

import numpy as np
import ml_dtypes

DEBUG_DUMPS = bool(os.environ.get("BASSDBG"))

sys.path.insert(0, "/opt/trn_rl_repo")

import concourse.bass as bass
import concourse.mybir as mybir
import concourse.tile as tile
from concourse import bacc, bass_utils
from concourse.masks import make_identity

F32 = mybir.dt.float32
F32R = mybir.dt.float32r
FP8 = mybir.dt.float8e4
AF = mybir.ActivationFunctionType
ALU = mybir.AluOpType
DR = mybir.MatmulPerfMode.DoubleRow

P = 128
S = 1024
D = 1024
H = 16
HD = 64
FF = 4096
ST = S // P   # 8
DT = D // P   # 8
FT = FF // P  # 32
NPAIR = H // 2
EPS = 1e-5
NLOG16 = -2.7725887


def fap(base, off, dims):
    """AP with base's partition dim, extra element offset, custom free dims."""
    return bass.AP(tensor=base.tensor, offset=base.offset + off,
                   ap=[list(base.ap[0])] + [list(d) for d in dims])


def build_program():
    nc = bacc.Bacc("TRN2", target_bir_lowering=False, debug=False)

    x = nc.dram_tensor("x", [S, D], mybir.dt.bfloat16, kind="ExternalInput").ap()
    wqkv8 = nc.dram_tensor("wqkv8", [P, DT, 3 * D], FP8, kind="ExternalInput").ap()
    qk_bias = nc.dram_tensor("qk_bias", [P, 2, NPAIR], F32, kind="ExternalInput").ap()
    vbias8 = nc.dram_tensor("vbias8", [1, D], FP8, kind="ExternalInput").ap()
    ones_aux8 = nc.dram_tensor("ones_aux8", [1, 2 * P], FP8, kind="ExternalInput").ap()
    wout8 = nc.dram_tensor("wout8", [HD, H, D], FP8, kind="ExternalInput").ap()
    bout8 = nc.dram_tensor("bout8", [1, D], FP8, kind="ExternalInput").ap()
    b1_col = nc.dram_tensor("b1_col", [P, FT], F32, kind="ExternalInput").ap()
    wpack1 = nc.dram_tensor("wpack1", [16, P, DT, 2, 256], FP8,
                            kind="ExternalInput").ap()
    w2pack8 = nc.dram_tensor("w2pack8", [4, P, FT, 2, 256], FP8,
                             kind="ExternalInput").ap()
    b2_8 = nc.dram_tensor("b2_8", [1, D], FP8, kind="ExternalInput").ap()
    out = nc.dram_tensor("out", [S, D], F32, kind="ExternalOutput").ap()
    if DEBUG_DUMPS:
        d_y1 = nc.dram_tensor("d_y1", [P, DT, S], FP8, kind="ExternalOutput").ap()
        d_vext = nc.dram_tensor("d_vext", [P, ST, H, HD + 1], FP8,
                                kind="ExternalOutput").ap()
        d_qT = nc.dram_tensor("d_qT", [P, S], FP8, kind="ExternalOutput").ap()
        d_kT = nc.dram_tensor("d_kT", [P, 2, S], FP8, kind="ExternalOutput").ap()
        d_oT = nc.dram_tensor("d_oT", [HD, H, S], FP8, kind="ExternalOutput").ap()
        d_x2 = nc.dram_tensor("d_x2", [P, ST, D], F32, kind="ExternalOutput").ap()
        d_y2h = nc.dram_tensor("d_y2h", [P, DT, 2, S], FP8, kind="ExternalOutput").ap()
        d_h1 = nc.dram_tensor("d_h1", [P, FT, 3, 512], FP8,
                              kind="ExternalOutput").ap()

    with tile.TileContext(nc) as tc, contextlib.ExitStack() as ctx:
        singles = ctx.enter_context(tc.tile_pool(name="singles", bufs=1))
        bigpool = ctx.enter_context(tc.tile_pool(name="bigpool", bufs=1))
        outp = ctx.enter_context(tc.tile_pool(name="outp", bufs=2))
        dram = ctx.enter_context(tc.tile_pool(name="dram", bufs=1, space="DRAM"))

        # ---- constants / small aux ----
        ident = singles.tile([P, P], F32)
        make_identity(nc, ident)
        identr = singles.tile([P, P], F32R)
        nc.vector.tensor_copy(out=identr, in_=ident)
        eps_t = singles.tile([P, 1], F32)
        nc.vector.memset(eps_t, EPS)
        nbias_t = singles.tile([P, 1], F32)
        nc.vector.memset(nbias_t, NLOG16)
        c32_t = singles.tile([P, 1], F32)
        nc.vector.memset(c32_t, 1.0 / 32.0)
        cq_t = singles.tile([P, 1], F32)
        nc.vector.memset(cq_t, 0.25)
        cm1_t = singles.tile([P, 1], F32)
        nc.vector.memset(cm1_t, -1.0)
        onesz = singles.tile([1, 2, P], FP8)          # [ones(128), zeros(128)]
        nc.sync.dma_start(onesz, ones_aux8.rearrange("o (t p) -> o t p", t=2))
        vb8 = singles.tile([1, D], FP8)
        nc.sync.dma_start(vb8, vbias8)
        bo8 = singles.tile([1, D], FP8)
        nc.sync.dma_start(bo8, bout8)
        bb2 = singles.tile([1, D], FP8)
        nc.sync.dma_start(bb2, b2_8)
        qkb = singles.tile([P, 2, NPAIR], F32)
        nc.sync.dma_start(qkb, qk_bias)
        b1c = singles.tile([P, FT], F32)
        nc.sync.dma_start(b1c, b1_col)


        def bias_mm(ps_ap, row8, c0, n, start, stop):
            """psum[:, :] += ones^T x bias_row chunk via K=1 DoubleRow inst."""
            rhs = fap(row8[0:1], c0, [[0, 2], [1, n]])
            nc.tensor.matmul(ps_ap, lhsT=onesz, rhs=rhs, start=start, stop=stop,
                             perf_mode=DR, skip_group_check=True)

        # ---- Phase A: LN1 -> y1 (fp8, feature-major [d-part, dt, slot, s];
        # slot 1 unused until LN2 reuses this tile for (y2h, y2h/32)) ----
        y1 = bigpool.tile([P, DT, 2, S], FP8, tag="y1")

        a_ps_ctx = contextlib.ExitStack()
        a_ps = a_ps_ctx.enter_context(tc.tile_pool(name="a_ps", bufs=2, space="PSUM"))
        sc_ps_ctx = contextlib.ExitStack()
        sc_ps = sc_ps_ctx.enter_context(tc.tile_pool(name="sc_ps", bufs=2, space="PSUM"))

        def ln_step(st, x_row, yh, yl, ps_pool, ps_tag, ln, slot1_scaled=False):
            stats = ln.tile([P, 2, 6], F32, tag="stats")
            xg = x_row.rearrange("p (n f) -> p n f", f=512)
            for g in range(2):
                nc.vector.bn_stats(out=stats[:, g, :], in_=xg[:, g, :])
            mv = ln.tile([P, 2], F32, tag="mv")
            nc.vector.bn_aggr(out=mv, in_=stats)
            rstd = ln.tile([P, 1], F32, tag="rstd")
            nc.scalar.activation(out=rstd, in_=mv[:, 1:2], func=AF.Sqrt,
                                 bias=eps_t, scale=1.0)
            nc.vector.reciprocal(out=rstd, in_=rstd)
            negms = ln.tile([P, 1], F32, tag="negms")
            nc.vector.tensor_scalar(out=negms, in0=mv[:, 0:1], scalar1=rstd,
                                    scalar2=cm1_t, op0=ALU.mult, op1=ALU.mult)
            y = ln.tile([P, D], F32R, tag="y")
            nc.scalar.activation(out=y, in_=x_row, func=AF.Identity,
                                 scale=rstd, bias=negms)
            for dg in range(2):
                ps = ps_pool.tile([P, 4, P], F32, tag=ps_tag, name="tp_ps")
                for j in range(4):
                    dt = dg * 4 + j
                    nc.tensor.transpose(ps[:, j, :].bitcast(F32R),
                                        y[:, dt * P:(dt + 1) * P],
                                        identr)
                # one copy for 4 transposed blocks: out dims (dt, s-col)
                oap = fap(yh[:, 0, 0, 0:1], (dg * 4) * 2 * S + st * P,
                          [[2 * S, 4], [1, P]])
                nc.vector.tensor_copy(out=oap, in_=ps)
                if slot1_scaled or yl is not None:
                    o32 = fap(yh[:, 0, 0, 0:1], (dg * 4) * 2 * S + S + st * P,
                              [[2 * S, 4], [1, P]])
                    nc.vector.tensor_scalar(out=o32, in0=ps, scalar1=c32_t,
                                            scalar2=None, op0=ALU.mult)
                if yl is not None:
                    lap = fap(yl[:, 0, 0:1], (dg * 4) * S + st * P,
                              [[S, 4], [1, P]])
                    nc.vector.tensor_tensor(out=lap, in0=ps, in1=oap,
                                            op=ALU.subtract)

        def ln_phase(x_rows, yh, yl, ps_pool, ps_tag):
            with contextlib.ExitStack() as sctx:
                ln = sctx.enter_context(tc.tile_pool(name="ln", bufs=4))
                for st in range(ST):
                    ln_step(st, x_rows(sctx, st), yh, yl, ps_pool, ps_tag, ln)

        # attention-lifetime pool (closed after phase D): weights + v + oT
        cd_ctx = contextlib.ExitStack()
        cd = cd_ctx.enter_context(tc.tile_pool(name="cd", bufs=1))
        wq8 = cd.tile([P, DT, 3 * D], FP8, tag="wq8")
        wo8 = cd.tile([HD, H, D], FP8, tag="wo8")

        # preload x rows; big weight loads issued after row 1 so the first
        # rows win the (serialized) DMA-engine resource and LN1 starts early
        xload_ctx = contextlib.ExitStack()
        xload = xload_ctx.enter_context(tc.tile_pool(name="xload", bufs=1))
        x_rows_t = []
        for st in range(ST):
            t = xload.tile([P, D], mybir.dt.bfloat16, tag=f"x{st}", name=f"x{st}")
            nc.gpsimd.dma_start(t, x[st * P:(st + 1) * P, :])
            x_rows_t.append(t)
            if st == 3:
                nc.sync.dma_start(wq8, wqkv8)
            if st == 5:
                nc.sync.dma_start(wo8, wout8)

        tp1_ctx = contextlib.ExitStack()
        tp1_ps = tp1_ctx.enter_context(tc.tile_pool(name="tp1_ps", bufs=2, space="PSUM"))
        ln_phase(lambda sctx, st: x_rows_t[st], y1, None, tp1_ps, "tp")

        xload_ctx.close()

        # ---- Phase B: V projection (natural [s-part, h, hd+1], fp8) ----
        # v and the appended ones column are scaled by 1/4 so the
        # unnormalized P@V output stays below fp8 max (240); the
        # denominator picks up the same factor, so normalization cancels it.
        v_ext = cd.tile([P, ST, H, HD + 1], FP8, tag="vx")
        nc.vector.memset(v_ext[:, :, :, HD:HD + 1], 0.25)
        for vc in range(2):
            for it in range(ST):
                ps = a_ps.tile([P, 512], F32, tag="proj")
                for qc in range(2):
                    pv = ps[:, qc * 256:(qc + 1) * 256]
                    for kp in range(4):
                        lhs = fap(y1[:, 0, 0, 0:1], (2 * kp) * 2 * S + it * P,
                                  [[2 * S, 2], [1, P]])
                        rhs = fap(wq8[:, 0, 0:1], (2 * kp) * 3 * D + vc * 512 + qc * 256,
                                  [[3 * D, 2], [1, 256]])
                        nc.tensor.matmul(pv, lhsT=lhs, rhs=rhs, start=(kp == 0),
                                         stop=False, perf_mode=DR,
                                         skip_group_check=True)
                    bias_mm(pv, vb8, vc * 512 + qc * 256, 256, False, True)
                oap = fap(v_ext[:, 0, 0, 0:1], it * H * (HD + 1) + vc * 8 * (HD + 1),
                          [[HD + 1, 8], [1, HD]])
                nc.vector.tensor_scalar(out=oap,
                                        in0=ps.rearrange("p (h c) -> p h c", c=HD),
                                        scalar1=cq_t, scalar2=None, op0=ALU.mult)
        tp1_ctx.close()
        if DEBUG_DUMPS:
            nc.sync.dma_start(d_y1, y1[:, :, 0, :])
            nc.sync.dma_start(d_vext, v_ext)

        # ---- Phase C: attention per head pair ----
        oT = cd.tile([HD, H, S], FP8, tag="oT")
        recip_dram = dram.tile([H, 2, 512], F32)
        qk_ctx = contextlib.ExitStack()
        qkp = qk_ctx.enter_context(tc.tile_pool(name="qkp", bufs=2))
        ptp = qk_ctx.enter_context(tc.tile_pool(name="ptp", bufs=5))
        rsp = qk_ctx.enter_context(tc.tile_pool(name="rsp", bufs=4))
        rbcp = qk_ctx.enter_context(tc.tile_pool(name="rbcp", bufs=2))
        ot_ctx = contextlib.ExitStack()
        ot_ps_pool = ot_ctx.enter_context(tc.tile_pool(name="ot_ps", bufs=2, space="PSUM"))

        # kTz buffers: [k-feat 128(2 heads), slot 2, s]; slot1 stays zero
        kTz = [cd.tile([P, 2, S], FP8, tag=f"kTz{i}", name=f"kTz{i}") for i in range(2)]
        qT = [cd.tile([P, S], FP8, tag=f"qT{i}", name=f"qT{i}") for i in range(2)]
        for i in range(2):
            nc.vector.memset(kTz[i][:, 1, :], 0.0)

        for p in range(NPAIR):
            qt_t, kt_t = qT[p % 2], kTz[p % 2]
            # Q/K projections: out [feat 128, s 512] per sh
            for c2 in range(2):  # 0 = q, 1 = k
                col0 = (1 + c2) * D + p * P
                for sh in range(2):
                    ps = a_ps.tile([P, 512], F32, tag="proj")
                    for qc in range(2):
                        pv = ps[:, qc * 256:(qc + 1) * 256]
                        for kp in range(4):
                            lhs = fap(wq8[:, 0, 0:1], (2 * kp) * 3 * D + col0,
                                      [[3 * D, 2], [1, P]])
                            rhs = fap(y1[:, 0, 0, 0:1],
                                      (2 * kp) * 2 * S + sh * 512 + qc * 256,
                                      [[2 * S, 2], [1, 256]])
                            nc.tensor.matmul(pv, lhsT=lhs, rhs=rhs, start=(kp == 0),
                                             stop=(kp == 3), perf_mode=DR,
                                             skip_group_check=True)
                    if c2 == 0:
                        dst = qt_t[:, sh * 512:(sh + 1) * 512]
                    else:
                        dst = kt_t[:, 0, sh * 512:(sh + 1) * 512]
                    nc.vector.tensor_scalar(out=dst, in0=ps,
                                            scalar1=qkb[:, c2, p:p + 1],
                                            scalar2=None, op0=ALU.add)
            for qt in range(2):
                ot_ps = [ot_ps_pool.tile([HD + 1, 512], F32, tag="ot",
                                         name=f"ot{e}") for e in range(2)]
                for e in range(2):
                    h = 2 * p + e
                    pts = []
                    for jc in range(4):
                        ssc = sc_ps.tile([P, 2, 512], F32, tag="sc")
                        for jj in range(2):
                            jt = jc * 2 + jj
                            for qc in range(2):
                                lhs = fap(kt_t[e * HD:(e + 1) * HD, 0, 0:1],
                                          jt * P, [[S, 2], [1, P]])
                                rhs = fap(qt_t[e * HD:(e + 1) * HD, 0:1],
                                          qt * 512 + qc * 256, [[0, 2], [1, 256]])
                                nc.tensor.matmul(
                                    ssc[:, jj, qc * 256:(qc + 1) * 256],
                                    lhsT=lhs, rhs=rhs, start=True, stop=True,
                                    perf_mode=DR, skip_group_check=True)
                        pt = ptp.tile([P, 2, 512], FP8, tag="pT")
                        nc.scalar.activation(out=pt, in_=ssc, func=AF.Exp,
                                             scale=0.125, bias=nbias_t)
                        pts.append(pt)
                    # full accumulation per 256-chunk (zero regions are 2KB:
                    # chunk groups must not interleave within a bank)
                    for qc in range(2):
                        for jc in range(4):
                            lhs = fap(v_ext[:, 0, 0, 0:1],
                                      (jc * 2) * H * (HD + 1) + h * (HD + 1),
                                      [[H * (HD + 1), 2], [1, HD + 1]])
                            rhs = fap(pts[jc][:, 0, 0:1], qc * 256,
                                      [[512, 2], [1, 256]])
                            nc.tensor.matmul(
                                ot_ps[e][:, qc * 256:(qc + 1) * 256],
                                lhsT=lhs, rhs=rhs, start=(jc == 0),
                                stop=(jc == 3), perf_mode=DR,
                                skip_group_check=True)
                for e in range(2):
                    h = 2 * p + e
                    nc.vector.tensor_copy(out=oT[:, h, qt * 512:(qt + 1) * 512],
                                          in_=ot_ps[e][0:HD, :])
                    rs = rsp.tile([1, 512], F32, tag="rs")
                    nc.vector.reciprocal(out=rs, in_=ot_ps[e][HD:HD + 1, :])
                    nc.sync.dma_start(
                        recip_dram.bitcast(F32)[h:h + 1, qt, :], rs)
                if p in (3, NPAIR - 1):
                    hb = (p - 3) // 4  # 0 or 1
                    rbc = rbcp.tile([HD, 8, 512], F32, tag="rbc")
                    src = bass.AP(
                        tensor=recip_dram.tensor,
                        offset=recip_dram.offset + hb * 8 * 1024 + qt * 512,
                        ap=[[0, HD], [1024, 8], [1, 512]])
                    nc.sync.dma_start(out=rbc, in_=src)
                    sl = oT[:, hb * 8:(hb + 1) * 8, qt * 512:(qt + 1) * 512]
                    nc.vector.tensor_tensor(out=sl, in0=sl, in1=rbc, op=ALU.mult)

        ot_ctx.close()
        sc_ps_ctx.close()
        a_ps_ctx.close()
        if DEBUG_DUMPS:
            nc.sync.dma_start(d_qT, qT[1])
            nc.sync.dma_start(d_kT, kTz[1])
            nc.sync.dma_start(d_oT, oT)

        qk_ctx.close()
        # ---- Phase D+E merged: out-proj/residual row, then LN2 of that
        # row immediately (interleaves LN2's DVE chain with D's adds) ----
        e_ps_ctx = contextlib.ExitStack()
        e_ps = e_ps_ctx.enter_context(tc.tile_pool(name="e_ps", bufs=2, space="PSUM"))
        d_ps_ctx = contextlib.ExitStack()
        d_ps = d_ps_ctx.enter_context(tc.tile_pool(name="d_ps", bufs=3, space="PSUM"))
        x2 = bigpool.tile([P, ST, D], F32, tag="x2")
        y2h = bigpool.tile([P, DT, 2, S], FP8, tag="y1")   # reuse y1 slot
        with tc.tile_pool(name="xrp", bufs=2) as xrp, \
                tc.tile_pool(name="ln2", bufs=4) as ln2p:
            for it in range(ST):
                for ct in range(2):
                    ps = d_ps.tile([P, 512], F32, tag="att")
                    for qc in range(2):
                        pv = ps[:, qc * 256:(qc + 1) * 256]
                        for hp in range(NPAIR):
                            lhs = fap(oT[:, 0, 0:1], (2 * hp) * S + it * P,
                                      [[S, 2], [1, P]])
                            rhs = fap(wo8[:, 0, 0:1], (2 * hp) * D + ct * 512 + qc * 256,
                                      [[D, 2], [1, 256]])
                            nc.tensor.matmul(pv, lhsT=lhs, rhs=rhs, start=(hp == 0),
                                             stop=False, perf_mode=DR,
                                             skip_group_check=True)
                        bias_mm(pv, bo8, ct * 512 + qc * 256, 256, False, True)
                    xr = xrp.tile([P, 512], mybir.dt.bfloat16, tag="xr")
                    nc.gpsimd.dma_start(xr, x[it * P:(it + 1) * P, ct * 512:(ct + 1) * 512])
                    nc.vector.tensor_add(out=x2[:, it, ct * 512:(ct + 1) * 512],
                                         in0=ps, in1=xr)
                ln_step(it, x2[:, it, :], y2h, None, e_ps, "tp", ln2p,
                        slot1_scaled=True)
        cd_ctx.close()
        d_ps_ctx.close()
        if DEBUG_DUMPS:
            nc.sync.dma_start(d_x2, x2)
            nc.sync.dma_start(d_y2h, y2h)

        # ---- Phase F: MLP per seq half.  MLP1 2-term (W hi/lo vs y2h);
        # MLP2 3-term: (h8, h8/32)x(w2h, 32*w2l) + unscaled-hl x w2h, with
        # gelu staged in bf16 so the three h slots are cheap DVE ops. ----
        with contextlib.ExitStack() as fctx:
            h1p = fctx.enter_context(tc.tile_pool(name="h1p", bufs=1))
            hfp = fctx.enter_context(tc.tile_pool(name="hfp", bufs=3))
            wch = fctx.enter_context(tc.tile_pool(name="wch", bufs=3))
            w2p = fctx.enter_context(tc.tile_pool(name="w2p", bufs=2))
            ps_m1 = fctx.enter_context(tc.tile_pool(name="ps_m1", bufs=2, space="PSUM"))
            ps_m2 = fctx.enter_context(tc.tile_pool(name="ps_m2", bufs=1, space="PSUM"))
            for sh in range(2):
                # h1: [ff-part 128, ft, slot3 (h8, h8/32, hl), s-half 512]
                h1 = h1p.tile([P, FT, 3, 512], FP8, tag="h1")
                for fc in range(16):   # stream w1 in 256-ff chunks
                    w1c = wch.tile([P, DT, 2, 256], FP8, tag="w1c")
                    nc.sync.dma_start(w1c, wpack1[fc])
                    for fl in range(2):
                        ft = fc * 2 + fl
                        ps = ps_m1.tile([P, 512], F32, tag="mlp1")
                        for qc in range(2):
                            pv = ps[:, qc * 256:(qc + 1) * 256]
                            for kt in range(DT):
                                # slot pair (w1h_k, 32*w1l_k) x (y2h_k, y2h_k/32)
                                lhsA = fap(w1c[:, 0, 0, 0:1], kt * 512 + fl * P,
                                           [[256, 2], [1, P]])
                                rhsA = fap(y2h[:, 0, 0, 0:1],
                                           kt * 2 * S + sh * 512 + qc * 256,
                                           [[S, 2], [1, 256]])
                                nc.tensor.matmul(pv, lhsT=lhsA, rhs=rhsA,
                                                 start=(kt == 0), stop=(kt == DT - 1),
                                                 perf_mode=DR, skip_group_check=True)
                        hf = hfp.tile([P, 512], mybir.dt.bfloat16, tag="hf")
                        nc.scalar.activation(out=hf, in_=ps, func=AF.Gelu,
                                             bias=b1c[:, ft:ft + 1], scale=1.0)
                        nc.vector.tensor_copy(out=h1[:, ft, 0, :], in_=hf)
                        nc.vector.tensor_scalar(out=h1[:, ft, 1, :],
                                                in0=h1[:, ft, 0, :],
                                                scalar1=c32_t, scalar2=None,
                                                op0=ALU.mult)
                        nc.vector.scalar_tensor_tensor(
                            out=h1[:, ft, 2, :], in0=h1[:, ft, 0, :],
                            scalar=-1.0, in1=hf, op0=ALU.mult, op1=ALU.add)
                if DEBUG_DUMPS and sh == 1:
                    nc.sync.dma_start(d_h1, h1)
                for ct in range(4):
                    mlp2_ps = [ps_m2.tile([P, 256], F32, tag=f"m2_{il}",
                                          name=f"m2_{il}", bufs=1) for il in range(4)]
                    for il in range(4):
                        bias_mm(mlp2_ps[il], bb2, ct * 256, 256, True, False)
                    for kh in range(2):   # stream w2 in FT/2-ktile halves
                        w2c = w2p.tile([P, FT // 2, 2, 256], FP8, tag="w2c")
                        (nc.sync if kh == 0 else nc.scalar).dma_start(
                            w2c, w2pack8[ct, :, kh * 16:(kh + 1) * 16, :, :])
                        for il in range(4):
                            pv = mlp2_ps[il]
                            for kl in range(FT // 2):
                                kt = kh * 16 + kl
                                # (h8_k, h8_k/32) x (w2h_k, 32*w2l_k)
                                lhsA = fap(h1[:, 0, 0, 0:1], kt * 1536 + il * P,
                                           [[512, 2], [1, P]])
                                rhsA = fap(w2c[:, 0, 0, 0:1], kl * 512,
                                           [[256, 2], [1, 256]])
                                nc.tensor.matmul(pv, lhsT=lhsA, rhs=rhsA,
                                                 start=False, stop=False,
                                                 perf_mode=DR, skip_group_check=True)
                            for kp in range(FT // 4):
                                kt0 = kh * 16 + 2 * kp
                                # (hl_k, hl_k1) x (w2h_k, w2h_k1)
                                lhsB = fap(h1[:, 0, 0, 0:1],
                                           kt0 * 1536 + 2 * 512 + il * P,
                                           [[1536, 2], [1, P]])
                                rhsB = fap(w2c[:, 0, 0, 0:1], (2 * kp) * 512,
                                           [[512, 2], [1, 256]])
                                nc.tensor.matmul(pv, lhsT=lhsB, rhs=rhsB,
                                                 start=False,
                                                 stop=(kh == 1 and kp == FT // 4 - 1),
                                                 perf_mode=DR, skip_group_check=True)
                    for il in range(4):
                        it = sh * 4 + il
                        ot = outp.tile([P, 256], F32, tag="fin")
                        nc.vector.tensor_add(out=ot, in0=mlp2_ps[il],
                                             in1=x2[:, it, ct * 256:(ct + 1) * 256])
                        nc.sync.dma_start(
                            out=out[it * P:(it + 1) * P, ct * 256:(ct + 1) * 256],
                            in_=ot)
        e_ps_ctx.close()

    nc.compile()
    return nc


_NC_CACHE = None


def _get_nc():
    global _NC_CACHE
    if _NC_CACHE is None:
        _NC_CACHE = build_program()
    return _NC_CACHE


def _q8(a):
    return a.astype(ml_dtypes.float8_e4m3)


def _prep_weights(inputs):
    f32 = lambda k: np.asarray(inputs[k], np.float32)
    ln1_g, ln1_b = f32("ln1_g"), f32("ln1_b")
    ln2_g, ln2_b = f32("ln2_g"), f32("ln2_b")
    w_qkv, w_out, b_out = f32("w_qkv"), f32("w_out"), f32("b_out")
    w1, b1, w2, b2 = f32("w1"), f32("b1"), f32("w2"), f32("b2")

    wqkv_g = w_qkv * ln1_g[:, None]
    wqkv8 = np.ascontiguousarray(
        _q8(wqkv_g).reshape(DT, P, 3 * D).transpose(1, 0, 2))
    bias_qkv = ln1_b @ w_qkv
    qk_bias = np.empty((P, 2, NPAIR), np.float32)
    for pp in range(NPAIR):
        qk_bias[:, 0, pp] = bias_qkv[D + pp * P:D + (pp + 1) * P]
        qk_bias[:, 1, pp] = bias_qkv[2 * D + pp * P:2 * D + (pp + 1) * P]
    vbias8 = _q8(bias_qkv[:D]).reshape(1, D)
    ones_aux = np.zeros((1, 2 * P), np.float32)
    ones_aux[0, :P] = 1.0
    wout8 = np.ascontiguousarray(
        _q8(w_out).reshape(H, HD, D).transpose(1, 0, 2))

    # lo words scaled x32 so they clear fp8's subnormal floor; the matmul
    # pairs them with x(1/32)-scaled activation copies.
    w1_g = w1 * ln2_g[:, None]
    w1h = _q8(w1_g)
    w1l = _q8(32.0 * (w1_g - w1h.astype(np.float32)))
    # [fc 16, P part, DT kt, 2 (hi, 32*lo), 256] -- chunk-contiguous in DRAM
    wpack1 = np.ascontiguousarray(np.stack(
        [w1h.reshape(DT, P, 16, 256).transpose(2, 1, 0, 3),
         w1l.reshape(DT, P, 16, 256).transpose(2, 1, 0, 3)], axis=3))
    bias1 = ln2_b @ w1 + b1
    b1_col = np.ascontiguousarray(bias1.reshape(FT, P).T)

    w2h = _q8(w2)
    w2l = _q8(32.0 * (w2 - w2h.astype(np.float32)))
    # [4 d-quarter, P part, FT kt, 2 (hi, 32*lo), 256]
    w2h8 = w2h.reshape(FT, P, 4, 256).transpose(2, 1, 0, 3)
    w2l8 = w2l.reshape(FT, P, 4, 256).transpose(2, 1, 0, 3)
    w2pack8 = np.ascontiguousarray(np.stack([w2h8, w2l8], axis=3))

    return {
        "wqkv8": wqkv8, "qk_bias": qk_bias, "vbias8": vbias8,
        "ones_aux8": _q8(ones_aux), "wout8": wout8,
        "bout8": _q8(b_out).reshape(1, D), "b1_col": b1_col,
        "wpack1": wpack1, "w2pack8": w2pack8,
        "b2_8": _q8(b2).reshape(1, D),
    }


WEIGHT_NAMES = ["wqkv8", "qk_bias", "vbias8", "ones_aux8", "wout8", "bout8",
                "b1_col", "wpack1", "w2pack8", "b2_8"]


def kernel(**inputs) -> np.ndarray:
    x = np.asarray(inputs["x"], dtype=np.float32).astype(ml_dtypes.bfloat16)
    B = x.shape[0]
    weights = _prep_weights(inputs)
    nc = _get_nc()
    in_maps = [{"x": np.ascontiguousarray(x[b]), **weights} for b in range(B)]
    res = bass_utils.run_bass_kernel_spmd(nc, in_maps, core_ids=list(range(B)))
    return np.stack([res.results[b]["out"] for b in range(B)], axis=0)



# revision 42
# speedup vs baseline: 1.0925x; 1.0260x over previous
"""Trainium2 Bass kernel for a dense transformer block (pre-LN, MHA + MLP).

Data-parallel over batch: 8 batch elements, one per NeuronCore; weights
replicated, no collectives.

All GEMMs run as fp8e4 (e4m3) DoubleRow matmuls (0.5 cycles/row vs 1.0 for
fp32r): one instruction contracts 2 k-tiles (up to 256).  Accuracy plan
(CPU-validated worst-case scale-rel err ~1.1e-2 vs the 2e-2 gate):
  - attention (QKV proj, scores, P@V, out proj): plain fp8 both operands.
  - MLP1/MLP2: 3-term hi/lo compensation: W = Wh+Wl (host-packed fp8 pair),
    activations a = ah+al (fp8 pair, residual computed on-device):
    W@a ~= (Wh+Wl)@ah + Wh@al  (drops only Wl@al ~ 2^-9).
  - LN gammas folded into the following weight matrices host-side; betas
    folded into bias rows (per-partition adds for q/k; fp8 ones-row matmul
    instructions for v / b_out / b2 -- exact for the zero biases actually
    used, 1-ulp-of-fp8 otherwise).
  - softmax: p = exp(s/8 - log 16) in fp8 (max ~25 < 240), denominators from
    an appended ones column in V so the normalizer matches quantized p.
Engine placement: exp/gelu on ScalarE; casts/adds/bn on DVE; oT normalize
mults on Pool; DMA triggers on SP (hwdge) and Pool (swdge).
"""
import contextlib
import os
import sys

import numpy as np
import ml_dtypes

DEBUG_DUMPS = bool(os.environ.get("BASSDBG"))

sys.path.insert(0, "/opt/trn_rl_repo")

import concourse.bass as bass
import concourse.mybir as mybir
import concourse.tile as tile
from concourse import bacc, bass_utils
from concourse.masks import make_identity

F32 = mybir.dt.float32
F32R = mybir.dt.float32r
FP8 = mybir.dt.float8e4
AF = mybir.ActivationFunctionType
ALU = mybir.AluOpType
DR = mybir.MatmulPerfMode.DoubleRow

P = 128
S = 1024
D = 1024
H = 16
HD = 64
FF = 4096
ST = S // P   # 8
DT = D // P   # 8
FT = FF // P  # 32
NPAIR = H // 2
EPS = 1e-5
NLOG16 = -2.7725887


def fap(base, off, dims):
    """AP with base's partition dim, extra element offset, custom free dims."""
    return bass.AP(tensor=base.tensor, offset=base.offset + off,
                   ap=[list(base.ap[0])] + [list(d) for d in dims])


def build_program():
    nc = bacc.Bacc("TRN2", target_bir_lowering=False, debug=False)

    x = nc.dram_tensor("x", [S, D], mybir.dt.bfloat16, kind="ExternalInput").ap()
    wqkv8 = nc.dram_tensor("wqkv8", [P, DT, 3 * D], FP8, kind="ExternalInput").ap()
    qk_bias = nc.dram_tensor("qk_bias", [P, 2, NPAIR], F32, kind="ExternalInput").ap()
    vbias8 = nc.dram_tensor("vbias8", [1, D], FP8, kind="ExternalInput").ap()
    ones_aux8 = nc.dram_tensor("ones_aux8", [1, 2 * P], FP8, kind="ExternalInput").ap()
    wout8 = nc.dram_tensor("wout8", [HD, H, D], FP8, kind="ExternalInput").ap()
    bout8 = nc.dram_tensor("bout8", [1, D], FP8, kind="ExternalInput").ap()
    b1_col = nc.dram_tensor("b1_col", [P, FT], F32, kind="ExternalInput").ap()
    wpack1 = nc.dram_tensor("wpack1", [16, P, DT, 2, 256], FP8,
                            kind="ExternalInput").ap()
    w2pack8 = nc.dram_tensor("w2pack8", [4, P, FT, 2, 256], FP8,
                             kind="ExternalInput").ap()
    b2_8 = nc.dram_tensor("b2_8", [1, D], FP8, kind="ExternalInput").ap()
    out = nc.dram_tensor("out", [S, D], F32, kind="ExternalOutput").ap()
    if DEBUG_DUMPS:
        d_y1 = nc.dram_tensor("d_y1", [P, DT, S], FP8, kind="ExternalOutput").ap()
        d_vext = nc.dram_tensor("d_vext", [P, ST, H, HD + 1], FP8,
                                kind="ExternalOutput").ap()
        d_qT = nc.dram_tensor("d_qT", [P, S], FP8, kind="ExternalOutput").ap()
        d_kT = nc.dram_tensor("d_kT", [P, 2, S], FP8, kind="ExternalOutput").ap()
        d_oT = nc.dram_tensor("d_oT", [HD, H, S], FP8, kind="ExternalOutput").ap()
        d_x2 = nc.dram_tensor("d_x2", [P, ST, D], F32, kind="ExternalOutput").ap()
        d_y2h = nc.dram_tensor("d_y2h", [P, DT, 2, S], FP8, kind="ExternalOutput").ap()
        d_h1 = nc.dram_tensor("d_h1", [P, FT, 3, 512], FP8,
                              kind="ExternalOutput").ap()

    with tile.TileContext(nc) as tc, contextlib.ExitStack() as ctx:
        singles = ctx.enter_context(tc.tile_pool(name="singles", bufs=1))
        bigpool = ctx.enter_context(tc.tile_pool(name="bigpool", bufs=1))
        outp = ctx.enter_context(tc.tile_pool(name="outp", bufs=2))
        dram = ctx.enter_context(tc.tile_pool(name="dram", bufs=1, space="DRAM"))

        # ---- constants / small aux ----
        ident = singles.tile([P, P], F32)
        make_identity(nc, ident)
        identr = singles.tile([P, P], F32R)
        nc.vector.tensor_copy(out=identr, in_=ident)
        eps_t = singles.tile([P, 1], F32)
        nc.vector.memset(eps_t, EPS)
        nbias_t = singles.tile([P, 1], F32)
        nc.vector.memset(nbias_t, NLOG16)
        c32_t = singles.tile([P, 1], F32)
        nc.vector.memset(c32_t, 1.0 / 32.0)
        cq_t = singles.tile([P, 1], F32)
        nc.vector.memset(cq_t, 0.25)
        cm1_t = singles.tile([P, 1], F32)
        nc.vector.memset(cm1_t, -1.0)
        onesz = singles.tile([1, 2, P], FP8)          # [ones(128), zeros(128)]
        nc.sync.dma_start(onesz, ones_aux8.rearrange("o (t p) -> o t p", t=2))
        vb8 = singles.tile([1, D], FP8)
        nc.sync.dma_start(vb8, vbias8)
        bo8 = singles.tile([1, D], FP8)
        nc.sync.dma_start(bo8, bout8)
        bb2 = singles.tile([1, D], FP8)
        nc.sync.dma_start(bb2, b2_8)
        qkb = singles.tile([P, 2, NPAIR], F32)
        nc.sync.dma_start(qkb, qk_bias)
        b1c = singles.tile([P, FT], F32)
        nc.sync.dma_start(b1c, b1_col)


        def bias_mm(ps_ap, row8, c0, n, start, stop):
            """psum[:, :] += ones^T x bias_row chunk via K=1 DoubleRow inst."""
            rhs = fap(row8[0:1], c0, [[0, 2], [1, n]])
            nc.tensor.matmul(ps_ap, lhsT=onesz, rhs=rhs, start=start, stop=stop,
                             perf_mode=DR, skip_group_check=True)

        # ---- Phase A: LN1 -> y1 (fp8, feature-major [d-part, dt, slot, s];
        # slot 1 unused until LN2 reuses this tile for (y2h, y2h/32)) ----
        y1 = bigpool.tile([P, DT, 2, S], FP8, tag="y1")

        a_ps_ctx = contextlib.ExitStack()
        a_ps = a_ps_ctx.enter_context(tc.tile_pool(name="a_ps", bufs=2, space="PSUM"))
        sc_ps_ctx = contextlib.ExitStack()
        sc_ps = sc_ps_ctx.enter_context(tc.tile_pool(name="sc_ps", bufs=2, space="PSUM"))

        def ln_step(st, x_row, yh, yl, ps_pool, ps_tag, ln, slot1_scaled=False):
            stats = ln.tile([P, 2, 6], F32, tag="stats")
            xg = x_row.rearrange("p (n f) -> p n f", f=512)
            for g in range(2):
                nc.vector.bn_stats(out=stats[:, g, :], in_=xg[:, g, :])
            mv = ln.tile([P, 2], F32, tag="mv")
            nc.vector.bn_aggr(out=mv, in_=stats)
            rstd = ln.tile([P, 1], F32, tag="rstd")
            nc.scalar.activation(out=rstd, in_=mv[:, 1:2], func=AF.Sqrt,
                                 bias=eps_t, scale=1.0)
            nc.vector.reciprocal(out=rstd, in_=rstd)
            negms = ln.tile([P, 1], F32, tag="negms")
            nc.vector.tensor_scalar(out=negms, in0=mv[:, 0:1], scalar1=rstd,
                                    scalar2=cm1_t, op0=ALU.mult, op1=ALU.mult)
            y = ln.tile([P, D], F32R, tag="y")
            nc.scalar.activation(out=y, in_=x_row, func=AF.Identity,
                                 scale=rstd, bias=negms)
            for dg in range(2):
                ps = ps_pool.tile([P, 4, P], F32, tag=ps_tag, name="tp_ps")
                for j in range(4):
                    dt = dg * 4 + j
                    nc.tensor.transpose(ps[:, j, :].bitcast(F32R),
                                        y[:, dt * P:(dt + 1) * P],
                                        identr)
                # one copy for 4 transposed blocks: out dims (dt, s-col)
                oap = fap(yh[:, 0, 0, 0:1], (dg * 4) * 2 * S + st * P,
                          [[2 * S, 4], [1, P]])
                nc.vector.tensor_copy(out=oap, in_=ps)
                if slot1_scaled or yl is not None:
                    o32 = fap(yh[:, 0, 0, 0:1], (dg * 4) * 2 * S + S + st * P,
                              [[2 * S, 4], [1, P]])
                    nc.vector.tensor_scalar(out=o32, in0=ps, scalar1=c32_t,
                                            scalar2=None, op0=ALU.mult)
                if yl is not None:
                    lap = fap(yl[:, 0, 0:1], (dg * 4) * S + st * P,
                              [[S, 4], [1, P]])
                    nc.vector.tensor_tensor(out=lap, in0=ps, in1=oap,
                                            op=ALU.subtract)

        def ln_phase(x_rows, yh, yl, ps_pool, ps_tag):
            with contextlib.ExitStack() as sctx:
                ln = sctx.enter_context(tc.tile_pool(name="ln", bufs=4))
                for st in range(ST):
                    ln_step(st, x_rows(sctx, st), yh, yl, ps_pool, ps_tag, ln)

        # attention-lifetime pool (closed after phase D): weights + v + oT
        cd_ctx = contextlib.ExitStack()
        cd = cd_ctx.enter_context(tc.tile_pool(name="cd", bufs=1))
        wq8 = cd.tile([P, DT, 3 * D], FP8, tag="wq8")
        wo8 = cd.tile([HD, H, D], FP8, tag="wo8")

        # preload x rows; big weight loads issued after row 1 so the first
        # rows win the (serialized) DMA-engine resource and LN1 starts early
        xload_ctx = contextlib.ExitStack()
        xload = xload_ctx.enter_context(tc.tile_pool(name="xload", bufs=1))
        x_rows_t = []
        for st in range(ST):
            t = xload.tile([P, D], mybir.dt.bfloat16, tag=f"x{st}", name=f"x{st}")
            nc.gpsimd.dma_start(t, x[st * P:(st + 1) * P, :])
            x_rows_t.append(t)
            if st == 3:
                nc.sync.dma_start(wq8, wqkv8)
            if st == 5:
                nc.sync.dma_start(wo8, wout8)

        tp1_ctx = contextlib.ExitStack()
        tp1_ps = tp1_ctx.enter_context(tc.tile_pool(name="tp1_ps", bufs=2, space="PSUM"))
        ln_phase(lambda sctx, st: x_rows_t[st], y1, None, tp1_ps, "tp")

        xload_ctx.close()

        # ---- Phase B: V projection (natural [s-part, h, hd+1], fp8) ----
        # v and the appended ones column are scaled by 1/4 so the
        # unnormalized P@V output stays below fp8 max (240); the
        # denominator picks up the same factor, so normalization cancels it.
        v_ext = cd.tile([P, ST, H, HD + 1], FP8, tag="vx")
        nc.vector.memset(v_ext[:, :, :, HD:HD + 1], 0.25)
        for vc in range(2):
            for it in range(ST):
                ps = a_ps.tile([P, 512], F32, tag="proj")
                for qc in range(2):
                    pv = ps[:, qc * 256:(qc + 1) * 256]
                    for kp in range(4):
                        lhs = fap(y1[:, 0, 0, 0:1], (2 * kp) * 2 * S + it * P,
                                  [[2 * S, 2], [1, P]])
                        rhs = fap(wq8[:, 0, 0:1], (2 * kp) * 3 * D + vc * 512 + qc * 256,
                                  [[3 * D, 2], [1, 256]])
                        nc.tensor.matmul(pv, lhsT=lhs, rhs=rhs, start=(kp == 0),
                                         stop=False, perf_mode=DR,
                                         skip_group_check=True)
                    bias_mm(pv, vb8, vc * 512 + qc * 256, 256, False, True)
                oap = fap(v_ext[:, 0, 0, 0:1], it * H * (HD + 1) + vc * 8 * (HD + 1),
                          [[HD + 1, 8], [1, HD]])
                nc.vector.tensor_scalar(out=oap,
                                        in0=ps.rearrange("p (h c) -> p h c", c=HD),
                                        scalar1=cq_t, scalar2=None, op0=ALU.mult)
        tp1_ctx.close()
        if DEBUG_DUMPS:
            nc.sync.dma_start(d_y1, y1[:, :, 0, :])
            nc.sync.dma_start(d_vext, v_ext)

        # ---- Phase C: attention per head pair ----
        oT = cd.tile([HD, H, S], FP8, tag="oT")
        recip_dram = dram.tile([H, 2, 512], F32)
        qk_ctx = contextlib.ExitStack()
        qkp = qk_ctx.enter_context(tc.tile_pool(name="qkp", bufs=2))
        ptp = qk_ctx.enter_context(tc.tile_pool(name="ptp", bufs=5))
        rsp = qk_ctx.enter_context(tc.tile_pool(name="rsp", bufs=4))
        rbcp = qk_ctx.enter_context(tc.tile_pool(name="rbcp", bufs=2))
        ot_ctx = contextlib.ExitStack()
        ot_ps_pool = ot_ctx.enter_context(tc.tile_pool(name="ot_ps", bufs=2, space="PSUM"))

        # kTz buffers: [k-feat 128(2 heads), slot 2, s]; slot1 stays zero
        kTz = [cd.tile([P, 2, S], FP8, tag=f"kTz{i}", name=f"kTz{i}") for i in range(2)]
        qT = [cd.tile([P, S], FP8, tag=f"qT{i}", name=f"qT{i}") for i in range(2)]
        for i in range(2):
            nc.vector.memset(kTz[i][:, 1, :], 0.0)

        for p in range(NPAIR):
            qt_t, kt_t = qT[p % 2], kTz[p % 2]
            # Q/K projections: out [feat 128, s 512] per sh
            for c2 in range(2):  # 0 = q, 1 = k
                col0 = (1 + c2) * D + p * P
                for sh in range(2):
                    ps = a_ps.tile([P, 512], F32, tag="proj")
                    for qc in range(2):
                        pv = ps[:, qc * 256:(qc + 1) * 256]
                        for kp in range(4):
                            lhs = fap(wq8[:, 0, 0:1], (2 * kp) * 3 * D + col0,
                                      [[3 * D, 2], [1, P]])
                            rhs = fap(y1[:, 0, 0, 0:1],
                                      (2 * kp) * 2 * S + sh * 512 + qc * 256,
                                      [[2 * S, 2], [1, 256]])
                            nc.tensor.matmul(pv, lhsT=lhs, rhs=rhs, start=(kp == 0),
                                             stop=(kp == 3), perf_mode=DR,
                                             skip_group_check=True)
                    if c2 == 0:
                        dst = qt_t[:, sh * 512:(sh + 1) * 512]
                    else:
                        dst = kt_t[:, 0, sh * 512:(sh + 1) * 512]
                    nc.vector.tensor_scalar(out=dst, in0=ps,
                                            scalar1=qkb[:, c2, p:p + 1],
                                            scalar2=None, op0=ALU.add)
            for qt in range(2):
                ot_ps = [ot_ps_pool.tile([HD + 1, 512], F32, tag="ot",
                                         name=f"ot{e}") for e in range(2)]
                for e in range(2):
                    h = 2 * p + e
                    pts = []
                    for jc in range(4):
                        ssc = sc_ps.tile([P, 2, 512], F32, tag="sc")
                        for jj in range(2):
                            jt = jc * 2 + jj
                            for qc in range(2):
                                lhs = fap(kt_t[e * HD:(e + 1) * HD, 0, 0:1],
                                          jt * P, [[S, 2], [1, P]])
                                rhs = fap(qt_t[e * HD:(e + 1) * HD, 0:1],
                                          qt * 512 + qc * 256, [[0, 2], [1, 256]])
                                nc.tensor.matmul(
                                    ssc[:, jj, qc * 256:(qc + 1) * 256],
                                    lhsT=lhs, rhs=rhs, start=True, stop=True,
                                    perf_mode=DR, skip_group_check=True)
                        pt = ptp.tile([P, 2, 512], FP8, tag="pT")
                        nc.scalar.activation(out=pt, in_=ssc, func=AF.Exp,
                                             scale=0.125, bias=nbias_t)
                        pts.append(pt)
                    # full accumulation per 256-chunk (zero regions are 2KB:
                    # chunk groups must not interleave within a bank)
                    for qc in range(2):
                        for jc in range(4):
                            lhs = fap(v_ext[:, 0, 0, 0:1],
                                      (jc * 2) * H * (HD + 1) + h * (HD + 1),
                                      [[H * (HD + 1), 2], [1, HD + 1]])
                            rhs = fap(pts[jc][:, 0, 0:1], qc * 256,
                                      [[512, 2], [1, 256]])
                            nc.tensor.matmul(
                                ot_ps[e][:, qc * 256:(qc + 1) * 256],
                                lhsT=lhs, rhs=rhs, start=(jc == 0),
                                stop=(jc == 3), perf_mode=DR,
                                skip_group_check=True)
                for e in range(2):
                    h = 2 * p + e
                    nc.vector.tensor_copy(out=oT[:, h, qt * 512:(qt + 1) * 512],
                                          in_=ot_ps[e][0:HD, :])
                    rs = rsp.tile([1, 512], F32, tag="rs")
                    nc.vector.reciprocal(out=rs, in_=ot_ps[e][HD:HD + 1, :])
                    nc.sync.dma_start(
                        recip_dram.bitcast(F32)[h:h + 1, qt, :], rs)
                if p in (3, NPAIR - 1):
                    hb = (p - 3) // 4  # 0 or 1
                    rbc = rbcp.tile([HD, 8, 512], F32, tag="rbc")
                    src = bass.AP(
                        tensor=recip_dram.tensor,
                        offset=recip_dram.offset + hb * 8 * 1024 + qt * 512,
                        ap=[[0, HD], [1024, 8], [1, 512]])
                    nc.sync.dma_start(out=rbc, in_=src)
                    sl = oT[:, hb * 8:(hb + 1) * 8, qt * 512:(qt + 1) * 512]
                    nc.vector.tensor_tensor(out=sl, in0=sl, in1=rbc, op=ALU.mult)

        ot_ctx.close()
        sc_ps_ctx.close()
        a_ps_ctx.close()
        if DEBUG_DUMPS:
            nc.sync.dma_start(d_qT, qT[1])
            nc.sync.dma_start(d_kT, kTz[1])
            nc.sync.dma_start(d_oT, oT)

        qk_ctx.close()
        # ---- Phase D+E merged: out-proj/residual row, then LN2 of that
        # row immediately (interleaves LN2's DVE chain with D's adds) ----
        e_ps_ctx = contextlib.ExitStack()
        e_ps = e_ps_ctx.enter_context(tc.tile_pool(name="e_ps", bufs=2, space="PSUM"))
        d_ps_ctx = contextlib.ExitStack()
        d_ps = d_ps_ctx.enter_context(tc.tile_pool(name="d_ps", bufs=3, space="PSUM"))
        x2 = bigpool.tile([P, ST, D], F32, tag="x2")
        y2h = bigpool.tile([P, DT, 2, S], FP8, tag="y1")   # reuse y1 slot
        with tc.tile_pool(name="xrp", bufs=2) as xrp, \
                tc.tile_pool(name="ln2", bufs=4) as ln2p:
            for it in range(ST):
                for ct in range(2):
                    ps = d_ps.tile([P, 512], F32, tag="att")
                    for qc in range(2):
                        pv = ps[:, qc * 256:(qc + 1) * 256]
                        for hp in range(NPAIR):
                            lhs = fap(oT[:, 0, 0:1], (2 * hp) * S + it * P,
                                      [[S, 2], [1, P]])
                            rhs = fap(wo8[:, 0, 0:1], (2 * hp) * D + ct * 512 + qc * 256,
                                      [[D, 2], [1, 256]])
                            nc.tensor.matmul(pv, lhsT=lhs, rhs=rhs, start=(hp == 0),
                                             stop=False, perf_mode=DR,
                                             skip_group_check=True)
                        bias_mm(pv, bo8, ct * 512 + qc * 256, 256, False, True)
                    xr = xrp.tile([P, 512], mybir.dt.bfloat16, tag="xr")
                    nc.gpsimd.dma_start(xr, x[it * P:(it + 1) * P, ct * 512:(ct + 1) * 512])
                    nc.gpsimd.tensor_add(out=x2[:, it, ct * 512:(ct + 1) * 512],
                                          in0=ps, in1=xr)
                ln_step(it, x2[:, it, :], y2h, None, e_ps, "tp", ln2p,
                        slot1_scaled=True)
        cd_ctx.close()
        d_ps_ctx.close()
        if DEBUG_DUMPS:
            nc.sync.dma_start(d_x2, x2)
            nc.sync.dma_start(d_y2h, y2h)

        # ---- Phase F: MLP per seq half.  MLP1 2-term (W hi/lo vs y2h);
        # MLP2 3-term: (h8, h8/32)x(w2h, 32*w2l) + unscaled-hl x w2h, with
        # gelu staged in bf16 so the three h slots are cheap DVE ops. ----
        with contextlib.ExitStack() as fctx:
            h1p = fctx.enter_context(tc.tile_pool(name="h1p", bufs=1))
            hfp = fctx.enter_context(tc.tile_pool(name="hfp", bufs=3))
            wch = fctx.enter_context(tc.tile_pool(name="wch", bufs=3))
            w2p = fctx.enter_context(tc.tile_pool(name="w2p", bufs=2))
            ps_m1 = fctx.enter_context(tc.tile_pool(name="ps_m1", bufs=2, space="PSUM"))
            ps_m2 = fctx.enter_context(tc.tile_pool(name="ps_m2", bufs=1, space="PSUM"))
            for sh in range(2):
                # h1: [ff-part 128, ft, slot3 (h8, h8/32, hl), s-half 512]
                h1 = h1p.tile([P, FT, 3, 512], FP8, tag="h1")
                for fc in range(16):   # stream w1 in 256-ff chunks
                    w1c = wch.tile([P, DT, 2, 256], FP8, tag="w1c")
                    nc.sync.dma_start(w1c, wpack1[fc])
                    for fl in range(2):
                        ft = fc * 2 + fl
                        ps = ps_m1.tile([P, 512], F32, tag="mlp1")
                        for qc in range(2):
                            pv = ps[:, qc * 256:(qc + 1) * 256]
                            for kt in range(DT):
                                # slot pair (w1h_k, 32*w1l_k) x (y2h_k, y2h_k/32)
                                lhsA = fap(w1c[:, 0, 0, 0:1], kt * 512 + fl * P,
                                           [[256, 2], [1, P]])
                                rhsA = fap(y2h[:, 0, 0, 0:1],
                                           kt * 2 * S + sh * 512 + qc * 256,
                                           [[S, 2], [1, 256]])
                                nc.tensor.matmul(pv, lhsT=lhsA, rhs=rhsA,
                                                 start=(kt == 0), stop=(kt == DT - 1),
                                                 perf_mode=DR, skip_group_check=True)
                        hf = hfp.tile([P, 512], mybir.dt.bfloat16, tag="hf")
                        nc.scalar.activation(out=hf, in_=ps, func=AF.Gelu,
                                             bias=b1c[:, ft:ft + 1], scale=1.0)
                        nc.vector.tensor_copy(out=h1[:, ft, 0, :], in_=hf)
                        nc.vector.tensor_scalar(out=h1[:, ft, 1, :],
                                                in0=h1[:, ft, 0, :],
                                                scalar1=c32_t, scalar2=None,
                                                op0=ALU.mult)
                        nc.vector.scalar_tensor_tensor(
                            out=h1[:, ft, 2, :], in0=h1[:, ft, 0, :],
                            scalar=-1.0, in1=hf, op0=ALU.mult, op1=ALU.add)
                if DEBUG_DUMPS and sh == 1:
                    nc.sync.dma_start(d_h1, h1)
                for ct in range(4):
                    mlp2_ps = [ps_m2.tile([P, 256], F32, tag=f"m2_{il}",
                                          name=f"m2_{il}", bufs=1) for il in range(4)]
                    for il in range(4):
                        bias_mm(mlp2_ps[il], bb2, ct * 256, 256, True, False)
                    for kh in range(2):   # stream w2 in FT/2-ktile halves
                        w2c = w2p.tile([P, FT // 2, 2, 256], FP8, tag="w2c")
                        (nc.sync if kh == 0 else nc.scalar).dma_start(
                            w2c, w2pack8[ct, :, kh * 16:(kh + 1) * 16, :, :])
                        for il in range(4):
                            pv = mlp2_ps[il]
                            for kl in range(FT // 2):
                                kt = kh * 16 + kl
                                # (h8_k, h8_k/32) x (w2h_k, 32*w2l_k)
                                lhsA = fap(h1[:, 0, 0, 0:1], kt * 1536 + il * P,
                                           [[512, 2], [1, P]])
                                rhsA = fap(w2c[:, 0, 0, 0:1], kl * 512,
                                           [[256, 2], [1, 256]])
                                nc.tensor.matmul(pv, lhsT=lhsA, rhs=rhsA,
                                                 start=False, stop=False,
                                                 perf_mode=DR, skip_group_check=True)
                            for kp in range(FT // 4):
                                kt0 = kh * 16 + 2 * kp
                                # (hl_k, hl_k1) x (w2h_k, w2h_k1)
                                lhsB = fap(h1[:, 0, 0, 0:1],
                                           kt0 * 1536 + 2 * 512 + il * P,
                                           [[1536, 2], [1, P]])
                                rhsB = fap(w2c[:, 0, 0, 0:1], (2 * kp) * 512,
                                           [[512, 2], [1, 256]])
                                nc.tensor.matmul(pv, lhsT=lhsB, rhs=rhsB,
                                                 start=False,
                                                 stop=(kh == 1 and kp == FT // 4 - 1),
                                                 perf_mode=DR, skip_group_check=True)
                    for il in range(4):
                        it = sh * 4 + il
                        ot = outp.tile([P, 256], F32, tag="fin")
                        nc.vector.tensor_add(out=ot, in0=mlp2_ps[il],
                                             in1=x2[:, it, ct * 256:(ct + 1) * 256])
                        nc.sync.dma_start(
                            out=out[it * P:(it + 1) * P, ct * 256:(ct + 1) * 256],
                            in_=ot)
        e_ps_ctx.close()

    nc.compile()
    return nc


_NC_CACHE = None


def _get_nc():
    global _NC_CACHE
    if _NC_CACHE is None:
        _NC_CACHE = build_program()
    return _NC_CACHE


def _q8(a):
    return a.astype(ml_dtypes.float8_e4m3)


def _prep_weights(inputs):
    f32 = lambda k: np.asarray(inputs[k], np.float32)
    ln1_g, ln1_b = f32("ln1_g"), f32("ln1_b")
    ln2_g, ln2_b = f32("ln2_g"), f32("ln2_b")
    w_qkv, w_out, b_out = f32("w_qkv"), f32("w_out"), f32("b_out")
    w1, b1, w2, b2 = f32("w1"), f32("b1"), f32("w2"), f32("b2")

    wqkv_g = w_qkv * ln1_g[:, None]
    wqkv8 = np.ascontiguousarray(
        _q8(wqkv_g).reshape(DT, P, 3 * D).transpose(1, 0, 2))
    bias_qkv = ln1_b @ w_qkv
    qk_bias = np.empty((P, 2, NPAIR), np.float32)
    for pp in range(NPAIR):
        qk_bias[:, 0, pp] = bias_qkv[D + pp * P:D + (pp + 1) * P]
        qk_bias[:, 1, pp] = bias_qkv[2 * D + pp * P:2 * D + (pp + 1) * P]
    vbias8 = _q8(bias_qkv[:D]).reshape(1, D)
    ones_aux = np.zeros((1, 2 * P), np.float32)
    ones_aux[0, :P] = 1.0
    wout8 = np.ascontiguousarray(
        _q8(w_out).reshape(H, HD, D).transpose(1, 0, 2))

    # lo words scaled x32 so they clear fp8's subnormal floor; the matmul
    # pairs them with x(1/32)-scaled activation copies.
    w1_g = w1 * ln2_g[:, None]
    w1h = _q8(w1_g)
    w1l = _q8(32.0 * (w1_g - w1h.astype(np.float32)))
    # [fc 16, P part, DT kt, 2 (hi, 32*lo), 256] -- chunk-contiguous in DRAM
    wpack1 = np.ascontiguousarray(np.stack(
        [w1h.reshape(DT, P, 16, 256).transpose(2, 1, 0, 3),
         w1l.reshape(DT, P, 16, 256).transpose(2, 1, 0, 3)], axis=3))
    bias1 = ln2_b @ w1 + b1
    b1_col = np.ascontiguousarray(bias1.reshape(FT, P).T)

    w2h = _q8(w2)
    w2l = _q8(32.0 * (w2 - w2h.astype(np.float32)))
    # [4 d-quarter, P part, FT kt, 2 (hi, 32*lo), 256]
    w2h8 = w2h.reshape(FT, P, 4, 256).transpose(2, 1, 0, 3)
    w2l8 = w2l.reshape(FT, P, 4, 256).transpose(2, 1, 0, 3)
    w2pack8 = np.ascontiguousarray(np.stack([w2h8, w2l8], axis=3))

    return {
        "wqkv8": wqkv8, "qk_bias": qk_bias, "vbias8": vbias8,
        "ones_aux8": _q8(ones_aux), "wout8": wout8,
        "bout8": _q8(b_out).reshape(1, D), "b1_col": b1_col,
        "wpack1": wpack1, "w2pack8": w2pack8,
        "b2_8": _q8(b2).reshape(1, D),
    }


WEIGHT_NAMES = ["wqkv8", "qk_bias", "vbias8", "ones_aux8", "wout8", "bout8",
                "b1_col", "wpack1", "w2pack8", "b2_8"]


def kernel(**inputs) -> np.ndarray:
    x = np.asarray(inputs["x"], dtype=np.float32).astype(ml_dtypes.bfloat16)
    B = x.shape[0]
    weights = _prep_weights(inputs)
    nc = _get_nc()
    in_maps = [{"x": np.ascontiguousarray(x[b]), **weights} for b in range(B)]
    res = bass_utils.run_bass_kernel_spmd(nc, in_maps, core_ids=list(range(B)))
    return np.stack([res.results[b]["out"] for b in range(B)], axis=0)

